# revision 25
# baseline (speedup 1.0000x reference)
"""Trainium2 Bass kernel for a 4-layer bigram-LM dense transformer.

Full-model shapes: B=2, T=2048, E=256, H=8, L=4, V=32000.

Sharding over 8 NeuronCores (self-contained, hardcoded):
  - 2-way data parallel over batch: cores 0-3 handle batch 0, cores 4-7
    batch 1 (a "batch group" of 4 cores each).
  - Within a batch group, per-token work (LN / QKV / wo / FFN) is
    replicated; attention (the exp-heavy part) is sharded 2 heads/core
    and re-assembled with one 4-rank AllGather per layer.
  - lm_head is sharded 4-way over vocab columns within the group
    (8000 cols/core, padded to 8192), so the dominant 524MB logits
    write is split 8 ways.

Compute layout: activations live transposed [E, T] in SBUF so every
matmul contracts over the partition axis with zero transposes. All
matmul operands are float32r (FP22 mantissa truncation, full PE rate
at moving-dim >= 256). Softmax skips the max-subtraction (scores are
~1e-1 scale; exp cannot overflow) and row sums ride along in the
attention-output matmul via a ones column packed next to V, with
normalization folded into the PSUM->SBUF copy.
"""

import numpy as np

import concourse.bass as bass
import concourse.mybir as mybir
import concourse.tile as tile
from concourse import bacc
from concourse.bass_utils import run_bass_kernel_spmd

AF = mybir.ActivationFunctionType
ALU = mybir.AluOpType
FP32 = mybir.dt.float32
FP32R = mybir.dt.float32r

# model dims (full problem)
B, T, E, H, L, V = 2, 2048, 256, 8, 4, 32000
HD = E // H  # 32
EPS = 1e-5
NCORES = 8
GROUP = 4  # cores per batch group
HPC = H // GROUP  # heads per core (2)
VS = V // GROUP  # vocab shard per core (8000)
VSP = 8192  # padded vocab shard
TB = 512  # t-block (PSUM bank free dim)
SC = 128  # s-chunk (partition dim)
ET = E // 128  # embedding partition tiles (2)
FF = 4 * E  # 1024
UT = FF // 128  # ffn u-tiles (8)


def build_nc(t=T, layers=L, vsp=VSP, use_collective=True, ablate=()):
    """Build + compile the per-core Bass program (SPMD: same program, 8 cores)."""
    nt = t // TB      # t-blocks
    nsc = t // SC     # s-chunks
    ntc = t // 128    # t-chunks for lm head
    nvb = vsp // 512  # vocab blocks

    nc = bacc.Bacc("TRN2", num_devices=NCORES)

    # ---- DRAM parameters (per core) ----
    x0 = nc.declare_dram_parameter("x0", [E, t], FP32R, isOutput=False)
    wqkv = nc.declare_dram_parameter("wqkv", [layers, 128, ET, 6 * HD], FP32R, isOutput=False)
    wo_p = nc.declare_dram_parameter("wo", [layers, 128, ET, E], FP32R, isOutput=False)
    w1_p = nc.declare_dram_parameter("w1", [layers, 128, ET, FF], FP32R, isOutput=False)
    w2_p = nc.declare_dram_parameter("w2", [layers, 128, UT, E], FP32R, isOutput=False)
    vecs = nc.declare_dram_parameter("vecs", [layers, 128, 20], FP32, isOutput=False)
    fvec = nc.declare_dram_parameter("fvec", [128, 4], FP32, isOutput=False)
    whead = nc.declare_dram_parameter("whead", [128, ET, vsp], FP32R, isOutput=False)
    maskp = nc.declare_dram_parameter("mask", [SC, SC], FP32, isOutput=False)
    peye = nc.declare_dram_parameter("peye", [128, nt, nt], FP32R, isOutput=False)
    vtc = nc.declare_dram_parameter("vtc", [128, nsc, 2], FP32R, False)
    onesr = nc.declare_dram_parameter("onesr", [1, 128], FP32R, isOutput=False)
    selp = nc.declare_dram_parameter("selp", [nt, nt, 128], FP32R, isOutput=False)
    logits = nc.declare_dram_parameter("logits", [t, vsp], FP32, isOutput=True)

    # internal DRAM bounce buffers for the per-layer AllGather
    cc_in = [nc.dram_tensor(f"cc_in{l}", [HPC * HD, t], FP32R) for l in range(layers)]
    cc_out = [nc.dram_tensor(f"cc_out{l}", [GROUP * HPC * HD, t], FP32R) for l in range(layers)]
    groups = [[0, 1, 2, 3], [4, 5, 6, 7]]

    from contextlib import ExitStack
    with tile.TileContext(nc) as tc:
        with ExitStack() as _ctx:
            persist = _ctx.enter_context(tc.tile_pool(name="persist", bufs=1))
            wpool2 = _ctx.enter_context(tc.tile_pool(name="wpool2", bufs=2))
            wpool1 = _ctx.enter_context(tc.tile_pool(name="wpool1", bufs=1))
            actp = _ctx.enter_context(tc.tile_pool(name="actp", bufs=1))
            xlnp = _ctx.enter_context(tc.tile_pool(name="xlnp", bufs=2))
            bigp = _ctx.enter_context(tc.tile_pool(name="bigp", bufs=3))
            expp = _ctx.enter_context(tc.tile_pool(name="expp", bufs=3))
            smallp = _ctx.enter_context(tc.tile_pool(name="smallp", bufs=2))
            tmpp = _ctx.enter_context(tc.tile_pool(name="tmpp", bufs=2))
            lgp = _ctx.enter_context(tc.tile_pool(name="lgp", bufs=4))
            whp = _ctx.enter_context(tc.tile_pool(name="whp", bufs=2))
            dpool = _ctx.enter_context(tc.tile_pool(name="dpool", bufs=2, space="DRAM"))
            ps_a = _ctx.enter_context(tc.tile_pool(name="ps_a", bufs=2, space="PSUM"))
            ps_o = _ctx.enter_context(tc.tile_pool(name="ps_o", bufs=1, space="PSUM"))
            ps_m = _ctx.enter_context(tc.tile_pool(name="ps_m", bufs=2, space="PSUM"))
            ps_s = _ctx.enter_context(tc.tile_pool(name="ps_s", bufs=1, space="PSUM"))
            # ---- persistent tiles ----
            xT = [persist.tile([128, t], FP32R, tag=f"xT{e}", name=f"xT{e}") for e in range(ET)]
            for e in range(ET):
                nc.sync.dma_start(out=xT[e], in_=x0[128 * e : 128 * (e + 1), :])
            mask = persist.tile([SC, SC], FP32, tag="mask")
            nc.sync.dma_start(out=mask, in_=maskp[:, :])
            fv = persist.tile([128, 4], FP32, tag="fvec")
            nc.sync.dma_start(out=fv, in_=fvec[:, :])
            # v tile: per chunk cols = [vA(32) | ones | vB(32) | ones] so the
            # 33-wide per-head lhsT computes o rows 0:32 plus a row-sum row 32
            vt = persist.tile([128, nsc, 2 * (HD + 1)], FP32R, tag="vt")
            nc.sync.dma_start(out=vt[:, :, HD : HD + 1], in_=vtc[:, :, 0:1])
            nc.sync.dma_start(out=vt[:, :, 2 * HD + 1 : 2 * HD + 2], in_=vtc[:, :, 1:2])
            eyeblk = persist.tile([128, nt, nt], FP32R, tag="eyeblk")
            nc.sync.dma_start(out=eyeblk, in_=peye[:, :, :])
            ones1c = persist.tile([1, 128], FP32R, tag="ones1c")
            nc.sync.dma_start(out=ones1c, in_=onesr[:, :])
            selt = persist.tile([nt, nt, 128], FP32R, tag="selt")
            nc.sync.dma_start(out=selt, in_=selp[:, :, :])
            # own heads' normalized attention out, pre-AllGather, [32, t] each
            oTp = [persist.tile([HD, t], FP32R, tag=f"oTp{h}", name=f"oTp{h}") for h in range(HPC)]
            epst = persist.tile([128, 1], FP32, tag="epst")
            nc.vector.memset(epst, EPS)

            def layernorm(src, g_ap_of, b_ap_of, out_tiles):
                if "ln" in ablate:
                    for e in range(ET):
                        nc.scalar.activation(
                            out=out_tiles[e][:, :], in_=src[e][:, :], func=AF.Identity,
                            bias=b_ap_of(e), scale=g_ap_of(e),
                        )
                    return
                """src: list of ET [128, t] fp32r tiles -> out_tiles fp32r.

                Per-token stats via ones-matmuls into PSUM rows {0,32,64,96}
                (one per t-block), then (x*s + m2)*g + b with s=rstd,
                m2=-mean*rstd broadcast along partitions.
                """
                sq = [
                    bigp.tile([128, t], FP32R, tag="big", name=f"sq{e}")
                    for e in range(ET)
                ]
                for e in range(ET):
                    nc.vector.tensor_tensor(
                        out=sq[e], in0=src[e], in1=src[e], op=ALU.mult
                    )
                xs_ps = ps_s.tile([nt, TB], FP32, tag="stat_x")
                qs_ps = ps_s.tile([nt, TB], FP32, tag="stat_q")
                for tb in range(nt):
                    for e in range(ET):
                        nc.tensor.matmul(
                            xs_ps[:, :],
                            eyeblk[:, tb, :],
                            src[e][:, TB * tb : TB * (tb + 1)],
                            start=(tb == 0 and e == 0),
                            stop=(tb == nt - 1 and e == ET - 1),
                        )
                    for e in range(ET):
                        nc.tensor.matmul(
                            qs_ps[:, :],
                            eyeblk[:, tb, :],
                            sq[e][:, TB * tb : TB * (tb + 1)],
                            start=(tb == 0 and e == 0),
                            stop=(tb == nt - 1 and e == ET - 1),
                        )
                mean4 = smallp.tile([nt, TB], FP32, tag="mean4", name="mean4")
                msq4 = smallp.tile([nt, TB], FP32, tag="msq4", name="msq4")
                var4 = smallp.tile([nt, TB], FP32, tag="var4", name="var4")
                s4 = smallp.tile([nt, TB], FP32R, tag="s4", name="s4")
                xs_rows = xs_ps[:, :]
                qs_rows = qs_ps[:, :]
                nc.vector.tensor_scalar(mean4[:, :], xs_rows, 1.0 / E, None, ALU.mult)
                nc.vector.tensor_scalar(msq4[:, :], qs_rows, 1.0 / E, None, ALU.mult)
                nc.vector.tensor_tensor(
                    out=var4[:, :], in0=mean4[:, :], in1=mean4[:, :], op=ALU.mult
                )
                nc.vector.tensor_tensor(
                    out=var4[:, :], in0=msq4[:, :], in1=var4[:, :], op=ALU.subtract
                )
                nc.scalar.activation(out=var4[:, :], in_=var4[:, :], func=AF.Ln, bias=epst[0:nt, :])
                nc.scalar.activation(out=s4[:, :], in_=var4[:, :], func=AF.Exp, scale=-0.5)
                m24 = smallp.tile([nt, TB], FP32R, tag="msq4", name="m24")
                nc.vector.scalar_tensor_tensor(
                    out=m24[:, :], in0=mean4[:, :], scalar=-1.0, in1=s4[:, :],
                    op0=ALU.mult, op1=ALU.mult,
                )
                for tb in range(nt):
                    s_bc = ps_s.tile([128, TB], FP32, tag="stat_x", name="s_bc")
                    m_bc = ps_s.tile([128, TB], FP32, tag="stat_q", name="m_bc")
                    nc.tensor.matmul(
                        s_bc[:, :], selt[:, tb, :], s4[:, :],
                        start=True, stop=True,
                    )
                    nc.tensor.matmul(
                        m_bc[:, :], selt[:, tb, :], m24[:, :],
                        start=True, stop=True,
                    )
                    for e in range(ET):
                        tmp = tmpp.tile([128, TB], FP32, tag="lntmp")
                        nc.vector.tensor_tensor(
                            out=tmp,
                            in0=src[e][:, TB * tb : TB * (tb + 1)],
                            in1=s_bc[:, :], op=ALU.mult,
                        )
                        nc.vector.tensor_tensor(
                            out=tmp, in0=tmp, in1=m_bc[:, :], op=ALU.add,
                        )
                        nc.scalar.activation(
                            out=out_tiles[e][:, TB * tb : TB * (tb + 1)],
                            in_=tmp, func=AF.Identity,
                            bias=b_ap_of(e), scale=g_ap_of(e),
                        )

            # ================= layers =================
            for l in range(layers):
                wq_t = [wpool2.tile([128, 6 * HD], FP32R, tag=f"wqkv{e}", name=f"wqkv{e}") for e in range(ET)]
                wo_t = [wpool2.tile([128, E], FP32R, tag=f"wo{e}", name=f"wot{e}") for e in range(ET)]
                w1_t = [wpool1.tile([128, FF], FP32R, tag=f"w1{e}", name=f"w1t{e}") for e in range(ET)]
                w2_t = wpool1.tile([128, UT, E], FP32R, tag="w2")
                vec = wpool2.tile([128, 20], FP32, tag="vec")
                for e in range(ET):
                    nc.sync.dma_start(out=wq_t[e], in_=wqkv[l, :, e, :])
                    nc.sync.dma_start(out=wo_t[e], in_=wo_p[l, :, e, :])
                    nc.sync.dma_start(out=w1_t[e], in_=w1_p[l, :, e, :])
                nc.sync.dma_start(out=w2_t, in_=w2_p[l, :, :, :])
                nc.sync.dma_start(out=vec, in_=vecs[l, :, :])

                xln = [xlnp.tile([128, t], FP32R, tag=f"xln{e}", name=f"xln{e}") for e in range(ET)]
                layernorm(
                    xT,
                    g_ap_of=lambda e: vec[:, 0 + e : 1 + e],
                    b_ap_of=lambda e: vec[:, 2 + e : 3 + e],
                    out_tiles=xln,
                )

                qT = actp.tile([2 * HD, t], FP32R, tag="qT")
                kT = actp.tile([2 * HD, t], FP32R, tag="kT")
                for tb in range(nt):
                    tsl = slice(TB * tb, TB * (tb + 1))
                    qp = ps_m.tile([2 * HD, TB], FP32, tag="m")
                    for e in range(ET):
                        nc.tensor.matmul(
                            qp[:, :], wq_t[e][:, 0 : 2 * HD], xln[e][:, tsl],
                            start=(e == 0), stop=(e == ET - 1),
                        )
                    nc.any.tensor_copy(out=qT[:, tsl], in_=qp[:, :])
                    kp = ps_m.tile([2 * HD, TB], FP32, tag="m")
                    for e in range(ET):
                        nc.tensor.matmul(
                            kp[:, :], wq_t[e][:, 2 * HD : 4 * HD], xln[e][:, tsl],
                            start=(e == 0), stop=(e == ET - 1),
                        )
                    nc.any.tensor_copy(out=kT[:, tsl], in_=kp[:, :])
                    for i in range(4 * tb, 4 * tb + 4):
                        vp = ps_m.tile([128, 2 * HD], FP32, tag="m")
                        for e in range(ET):
                            nc.tensor.matmul(
                                vp[:, :],
                                xln[e][:, SC * i : SC * (i + 1)],
                                wq_t[e][:, 4 * HD : 6 * HD],
                                start=(e == 0), stop=(e == ET - 1),
                            )
                        nc.any.tensor_copy(out=vt[:, i, 0:HD], in_=vp[:, 0:HD])
                        nc.any.tensor_copy(
                            out=vt[:, i, HD + 1 : 2 * HD + 1], in_=vp[:, HD : 2 * HD]
                        )

                    # ---- attention for this t-block (qkv ready up to here) ----
                    if "attn" in ablate:
                        if tb == 0:
                            for h in range(HPC):
                                nc.vector.memset(oTp[h].bitcast(FP32), 1.0)
                        continue
                    op_ps = [
                        ps_o.tile([HD + 1, TB], FP32, tag=f"o{h}", name=f"op_ps{h}")
                        for h in range(HPC)
                    ]
                    nmax = 4 * tb + 4
                    for h in range(HPC):
                        rsl = slice(32 * h, 32 * (h + 1))

                        def emit_o(i, exh, d):
                            nc.tensor.matmul(
                                op_ps[h][:, d:TB],
                                vt[:, i, (HD + 1) * h : (HD + 1) * h + HD + 1],
                                exh[:, d:TB],
                                start=(i == 0), stop=(i == nmax - 1),
                            )

                        pend = None
                        for i in range(nmax):
                            d = max(0, SC * i - TB * tb)
                            psl = slice(d, TB)
                            tgl = slice(TB * tb + d, TB * (tb + 1))
                            at_ps = ps_a.tile([128, TB], FP32, tag="att", name="at_ps")
                            exh = expp.tile([128, TB], FP32R, tag=f"exp{h}", name="exh")
                            nc.tensor.matmul(
                                at_ps[:, psl],
                                kT[rsl, SC * i : SC * (i + 1)],
                                qT[rsl, tgl],
                                start=True, stop=True,
                                tile_position=(32 * h, 0),
                            )
                            nc.scalar.activation(
                                out=exh[:, psl], in_=at_ps[:, psl],
                                func=AF.Exp, scale=float(E) ** -0.5,
                            )
                            if i >= 4 * tb:  # diagonal chunk: mask upper triangle
                                nc.vector.tensor_tensor(
                                    out=exh[:, d : d + SC],
                                    in0=exh[:, d : d + SC],
                                    in1=mask[:, :], op=ALU.mult,
                                )
                            if pend is not None:
                                emit_o(*pend)
                            pend = (i, exh, d)
                        emit_o(*pend)
                    # normalize each head by its row-sum (psum row 32)
                    srow = [
                        smallp.tile([HD + 1, TB], FP32, tag=f"srow{h}", name=f"srow{h}")
                        for h in range(HPC)
                    ]
                    rd = dpool.tile([HPC, TB], FP32, tag="rd", name="rd")
                    for h in range(HPC):
                        nc.vector.reciprocal(
                            out=srow[h][HD : HD + 1, :],
                            in_=op_ps[h][HD : HD + 1, :],
                        )
                        nc.sync.dma_start(
                            out=rd[h : h + 1, :],
                            in_=srow[h][HD : HD + 1, :],
                        )
                    rec_bc = tmpp.tile([HD, TB], FP32, tag="rbc", name="rec_bc")
                    for h in range(HPC):
                        nc.gpsimd.dma_start(
                            out=rec_bc,
                            in_=rd[h : h + 1, :].partition_broadcast(HD),
                        )
                        nc.vector.tensor_tensor(
                            out=oTp[h][:, TB * tb : TB * (tb + 1)],
                            in0=op_ps[h][0:HD, :],
                            in1=rec_bc,
                            op=ALU.mult,
                        )

                # ---- AllGather heads across the 4-core batch group ----
                oT = [actp.tile([128, t], FP32R, tag=tg, name=f"oT_{tg}") for tg in ("qT", "kT")]
                if use_collective:
                    for h in range(HPC):
                        nc.sync.dma_start(
                            out=cc_in[l][HD * h : HD * (h + 1), :], in_=oTp[h][:, :]
                        )
                    nc.gpsimd.collective_compute(
                        "AllGather", ALU.bypass,
                        replica_groups=groups,
                        ins=[cc_in[l][:, :]], outs=[cc_out[l][:, :]],
                    )
                    for e in range(ET):
                        nc.sync.dma_start(
                            out=oT[e], in_=cc_out[l][128 * e : 128 * (e + 1), :]
                        )
                else:  # single-group debug path (no comm): own heads only
                    for e in range(ET):
                        nc.vector.memset(oT[e].bitcast(FP32), 0.0)
                    for h in range(HPC):
                        nc.vector.tensor_copy(
                            out=oT[0][HD * h : HD * (h + 1), :].bitcast(FP32),
                            in_=oTp[h][:, :].bitcast(FP32),
                        )

                # ---- wo projection + residual ----
                for tb in range(nt):
                    tsl = slice(TB * tb, TB * (tb + 1))
                    for eo in range(ET):
                        wp = ps_m.tile([128, TB], FP32, tag="m")
                        for e in range(ET):
                            nc.tensor.matmul(
                                wp[:, :],
                                wo_t[e][:, 128 * eo : 128 * (eo + 1)],
                                oT[e][:, tsl],
                                start=(e == 0), stop=(e == ET - 1),
                            )
                        nc.vector.scalar_tensor_tensor(
                            out=xT[eo][:, tsl], in0=wp[:, :],
                            scalar=vec[:, 8 + eo : 9 + eo], in1=xT[eo][:, tsl],
                            op0=ALU.add, op1=ALU.add,
                        )

                # ---- FFN ----
                xln2 = [xlnp.tile([128, t], FP32R, tag=f"xln{e}", name=f"xln{e}") for e in range(ET)]
                layernorm(
                    xT,
                    g_ap_of=lambda e: vec[:, 4 + e : 5 + e],
                    b_ap_of=lambda e: vec[:, 6 + e : 7 + e],
                    out_tiles=xln2,
                )
                for tb in range(nt):
                    if "ffn" in ablate:
                        break
                    tsl = slice(TB * tb, TB * (tb + 1))
                    ru_halves = []
                    for half in range(2):
                        ru = bigp.tile([128, UT // 2, TB], FP32R, tag="big", name="ru")
                        for uu in range(UT // 2):
                            ut = half * (UT // 2) + uu
                            up = ps_a.tile([128, TB], FP32, tag="att", name="up")
                            for e in range(ET):
                                nc.tensor.matmul(
                                    up[:, :],
                                    w1_t[e][:, 128 * ut : 128 * (ut + 1)],
                                    xln2[e][:, tsl],
                                    start=(e == 0), stop=(e == ET - 1),
                                )
                            nc.scalar.activation(
                                out=ru[:, uu, :], in_=up[:, :], func=AF.Relu,
                                bias=vec[:, 10 + ut : 11 + ut],
                            )
                        ru_halves.append(ru)
                    for eo in range(ET):
                        wp2 = ps_m.tile([128, TB], FP32, tag="m", name="wp2")
                        for ut in range(UT):
                            nc.tensor.matmul(
                                wp2[:, :],
                                w2_t[:, ut, 128 * eo : 128 * (eo + 1)],
                                ru_halves[ut // (UT // 2)][:, ut % (UT // 2), :],
                                start=(ut == 0), stop=(ut == UT - 1),
                            )
                        nc.vector.scalar_tensor_tensor(
                            out=xT[eo][:, tsl], in0=wp2[:, :],
                            scalar=vec[:, 18 + eo : 19 + eo], in1=xT[eo][:, tsl],
                            op0=ALU.add, op1=ALU.add,
                        )

            # ================= final LN + lm_head =================
            xf = [xlnp.tile([128, t], FP32R, tag=f"xln{e}", name=f"xln{e}") for e in range(ET)]
            layernorm(
                xT,
                g_ap_of=lambda e: fv[:, 0 + e : 1 + e],
                b_ap_of=lambda e: fv[:, 2 + e : 3 + e],
                out_tiles=xf,
            )
            for vb in range(nvb if "lm" not in ablate else 1):
                wh = whp.tile([128, ET, 512], FP32R, tag="wh")
                nc.sync.dma_start(out=wh, in_=whead[:, :, 512 * vb : 512 * (vb + 1)])
                for tcn in range(ntc):
                    lp = ps_m.tile([128, 512], FP32, tag="m")
                    for e in range(ET):
                        nc.tensor.matmul(
                            lp[:, :],
                            xf[e][:, 128 * tcn : 128 * (tcn + 1)],
                            wh[:, e, :],
                            start=(e == 0), stop=(e == ET - 1),
                        )
                    lg = lgp.tile([128, 512], FP32, tag="lg")
                    if (vb + tcn) % 2 == 0:
                        nc.vector.tensor_copy(out=lg, in_=lp[:, :])
                    else:
                        nc.scalar.copy(out=lg, in_=lp[:, :])
                    nc.sync.dma_start(
                        out=logits[128 * tcn : 128 * (tcn + 1), 512 * vb : 512 * (vb + 1)],
                        in_=lg,
                    )

    nc.compile()
    return nc


# ---------------- host-side prep / unshard ----------------

def prep_core_inputs(c, X, tok_emb, pos_emb, wq, wk, wv, wo, bo, w1, b1, w2, b2,
                     ln1_g, ln1_b, ln2_g, ln2_b, lnf_g, lnf_b, w_head, b_head,
                     t=T, layers=L, vsp=VSP):
    b = c // GROUP
    j = c % GROUP
    heads = [HPC * j + k for k in range(HPC)]

    f32 = np.float32
    Xb = np.asarray(X[b]).astype(np.int64)
    x0 = (np.asarray(tok_emb)[Xb] + np.asarray(pos_emb)[:t]).astype(f32).T  # [E, t]

    wq = np.asarray(wq); wk = np.asarray(wk); wv = np.asarray(wv)
    wqkv_h = np.empty((layers, 128, ET, 6 * HD), f32)
    wo_h = np.empty((layers, 128, ET, E), f32)
    w1_h = np.empty((layers, 128, ET, FF), f32)
    w2_h = np.empty((layers, 128, UT, E), f32)
    vecs_h = np.empty((layers, 128, 20), f32)
    for l in range(layers):
        qc = np.concatenate([wq[l, h] for h in heads], axis=1)  # [E, 64]
        kc = np.concatenate([wk[l, h] for h in heads], axis=1)
        vc = np.concatenate([wv[l, h] for h in heads], axis=1)
        qkv = np.concatenate([qc, kc, vc], axis=1)  # [E, 192]
        wqkv_h[l] = qkv.reshape(ET, 128, 6 * HD).transpose(1, 0, 2)
        wo_h[l] = np.asarray(wo[l]).reshape(ET, 128, E).transpose(1, 0, 2)
        w1_h[l] = np.asarray(w1[l]).reshape(ET, 128, FF).transpose(1, 0, 2)
        w2_h[l] = np.asarray(w2[l]).reshape(UT, 128, E).transpose(1, 0, 2)
        vv = np.concatenate([
            np.asarray(ln1_g[l]), np.asarray(ln1_b[l]),
            np.asarray(ln2_g[l]), np.asarray(ln2_b[l]),
            np.asarray(bo[l]), np.asarray(b1[l]), np.asarray(b2[l]),
        ]).astype(f32)  # 2560
        vecs_h[l] = vv.reshape(20, 128).T
    fvec_h = np.concatenate(
        [np.asarray(lnf_g), np.asarray(lnf_b)]
    ).astype(f32).reshape(4, 128).T

    w_head = np.asarray(w_head)
    vs = w_head.shape[1] // GROUP
    wh = np.zeros((E, vsp), f32)
    wh[:, :vs] = w_head[:, vs * j : vs * (j + 1)]
    whead_h = np.ascontiguousarray(wh.reshape(ET, 128, vsp).transpose(1, 0, 2))

    sp = np.arange(SC)[:, None]
    tp = np.arange(SC)[None, :]
    mask_h = (sp <= tp).astype(f32)

    nt = t // TB
    nsc = t // SC
    peye_h = np.zeros((128, nt, nt), f32)
    for tb in range(nt):
        peye_h[:, tb, tb] = 1.0
    vtc_h = np.ones((128, nsc, 2), f32)

    return {
        "x0": np.ascontiguousarray(x0),
        "wqkv": np.ascontiguousarray(wqkv_h),
        "wo": np.ascontiguousarray(wo_h),
        "w1": np.ascontiguousarray(w1_h),
        "w2": np.ascontiguousarray(w2_h),
        "vecs": np.ascontiguousarray(vecs_h),
        "fvec": np.ascontiguousarray(fvec_h),
        "whead": whead_h,
        "mask": mask_h,
        "peye": peye_h,
        "vtc": vtc_h,
        "onesr": np.ones((1, 128), f32),
        "selp": np.ascontiguousarray(
            np.broadcast_to(np.eye(nt, dtype=f32)[:, :, None], (nt, nt, 128))
        ),
    }


_NC_CACHE = {}


def _get_nc():
    if "nc" not in _NC_CACHE:
        _NC_CACHE["nc"] = build_nc()
    return _NC_CACHE["nc"]


def kernel(**inputs):
    nc = _get_nc()
    in_maps = [prep_core_inputs(c, **inputs) for c in range(NCORES)]
    res = run_bass_kernel_spmd(nc, in_maps, list(range(NCORES)))
    out = np.empty((B, T, V), np.float32)
    for c in range(NCORES):
        b, j = c // GROUP, c % GROUP
        out[b, :, VS * j : VS * (j + 1)] = res.results[c]["logits"][:, :VS]
    b_head = np.asarray(inputs["b_head"])
    if np.any(b_head):
        out += b_head[None, None, :]
    return out


# revision 26
# speedup vs baseline: 30453.7040x; 30453.7040x over previous
"""Trainium2 Bass kernel for a 4-layer bigram-LM dense transformer.

Full-model shapes: B=2, T=2048, E=256, H=8, L=4, V=32000.

Sharding over 8 NeuronCores (self-contained, hardcoded):
  - 2-way data parallel over batch: cores 0-3 handle batch 0, cores 4-7
    batch 1 (a "batch group" of 4 cores each).
  - Within a batch group, per-token work (LN / QKV / wo / FFN) is
    replicated; attention (the exp-heavy part) is sharded 2 heads/core
    and re-assembled with one 4-rank AllGather per layer.
  - lm_head is sharded 4-way over vocab columns within the group
    (8000 cols/core, padded to 8192), so the dominant 524MB logits
    write is split 8 ways.

Compute layout: activations live transposed [E, T] in SBUF so every
matmul contracts over the partition axis with zero transposes. All
matmul operands are float32r (FP22 mantissa truncation, full PE rate
at moving-dim >= 256). Softmax skips the max-subtraction (scores are
~1e-1 scale; exp cannot overflow) and row sums ride along in the
attention-output matmul via a ones column packed next to V, with
normalization folded into the PSUM->SBUF copy.
"""

import numpy as np

import concourse.bass as bass
import concourse.mybir as mybir
import concourse.tile as tile
from concourse import bacc
from concourse.bass_utils import run_bass_kernel_spmd

AF = mybir.ActivationFunctionType
ALU = mybir.AluOpType
FP32 = mybir.dt.float32
FP32R = mybir.dt.float32r

# model dims (full problem)
B, T, E, H, L, V = 2, 2048, 256, 8, 4, 32000
HD = E // H  # 32
EPS = 1e-5
NCORES = 8
GROUP = 4  # cores per batch group
HPC = H // GROUP  # heads per core (2)
VS = V // GROUP  # vocab shard per core (8000)
VSP = 8192  # padded vocab shard
TB = 512  # t-block (PSUM bank free dim)
SC = 128  # s-chunk (partition dim)
ET = E // 128  # embedding partition tiles (2)
FF = 4 * E  # 1024
UT = FF // 128  # ffn u-tiles (8)


def build_nc(t=T, layers=L, vsp=VSP, use_collective=True, ablate=()):
    """Build + compile the per-core Bass program (SPMD: same program, 8 cores)."""
    nt = t // TB      # t-blocks
    nsc = t // SC     # s-chunks
    ntc = t // 128    # t-chunks for lm head
    nvb = vsp // 512  # vocab blocks

    nc = bacc.Bacc("TRN2", num_devices=NCORES)

    # ---- DRAM parameters (per core) ----
    x0 = nc.declare_dram_parameter("x0", [E, t], FP32R, isOutput=False)
    wqkv = nc.declare_dram_parameter("wqkv", [layers, 128, ET, 6 * HD], FP32R, isOutput=False)
    wo_p = nc.declare_dram_parameter("wo", [layers, 128, ET, E], FP32R, isOutput=False)
    w1_p = nc.declare_dram_parameter("w1", [layers, 128, ET, FF], FP32R, isOutput=False)
    w2_p = nc.declare_dram_parameter("w2", [layers, 128, UT, E], FP32R, isOutput=False)
    vecs = nc.declare_dram_parameter("vecs", [layers, 128, 20], FP32, isOutput=False)
    fvec = nc.declare_dram_parameter("fvec", [128, 4], FP32, isOutput=False)
    whead = nc.declare_dram_parameter("whead", [128, ET, vsp], FP32R, isOutput=False)
    maskp = nc.declare_dram_parameter("mask", [SC, SC], FP32, isOutput=False)
    peye = nc.declare_dram_parameter("peye", [128, nt, nt], FP32R, isOutput=False)
    vtc = nc.declare_dram_parameter("vtc", [128, nsc, 2], FP32R, False)
    onesr = nc.declare_dram_parameter("onesr", [1, 128], FP32R, isOutput=False)
    selp = nc.declare_dram_parameter("selp", [nt, nt, 128], FP32R, isOutput=False)
    logits = nc.declare_dram_parameter("logits", [t, vsp], FP32, isOutput=True)

    # internal DRAM bounce buffers for the per-layer AllGather
    cc_in = [nc.dram_tensor(f"cc_in{l}", [HPC * HD, t], FP32R) for l in range(layers)]
    cc_out = [nc.dram_tensor(f"cc_out{l}", [GROUP * HPC * HD, t], FP32R) for l in range(layers)]
    groups = [[0, 1, 2, 3], [4, 5, 6, 7]]

    from contextlib import ExitStack
    with tile.TileContext(nc) as tc:
        with ExitStack() as _ctx:
            persist = _ctx.enter_context(tc.tile_pool(name="persist", bufs=1))
            wpool2 = _ctx.enter_context(tc.tile_pool(name="wpool2", bufs=2))
            wpool1 = _ctx.enter_context(tc.tile_pool(name="wpool1", bufs=1))
            actp = _ctx.enter_context(tc.tile_pool(name="actp", bufs=1))
            xlnp = _ctx.enter_context(tc.tile_pool(name="xlnp", bufs=2))
            bigp = _ctx.enter_context(tc.tile_pool(name="bigp", bufs=3))
            expp = _ctx.enter_context(tc.tile_pool(name="expp", bufs=3))
            smallp = _ctx.enter_context(tc.tile_pool(name="smallp", bufs=2))
            tmpp = _ctx.enter_context(tc.tile_pool(name="tmpp", bufs=2))
            lgp = _ctx.enter_context(tc.tile_pool(name="lgp", bufs=6))
            whp = _ctx.enter_context(tc.tile_pool(name="whp", bufs=2))
            dpool = _ctx.enter_context(tc.tile_pool(name="dpool", bufs=2, space="DRAM"))
            ps_a = _ctx.enter_context(tc.tile_pool(name="ps_a", bufs=2, space="PSUM"))
            ps_o = _ctx.enter_context(tc.tile_pool(name="ps_o", bufs=1, space="PSUM"))
            ps_m = _ctx.enter_context(tc.tile_pool(name="ps_m", bufs=2, space="PSUM"))
            ps_s = _ctx.enter_context(tc.tile_pool(name="ps_s", bufs=1, space="PSUM"))
            # ---- persistent tiles ----
            xT = [persist.tile([128, t], FP32R, tag=f"xT{e}", name=f"xT{e}") for e in range(ET)]
            for e in range(ET):
                nc.sync.dma_start(out=xT[e], in_=x0[128 * e : 128 * (e + 1), :])
            mask = persist.tile([SC, SC], FP32, tag="mask")
            nc.sync.dma_start(out=mask, in_=maskp[:, :])
            fv = persist.tile([128, 4], FP32, tag="fvec")
            nc.sync.dma_start(out=fv, in_=fvec[:, :])
            # v tile: per chunk cols = [vA(32) | ones | vB(32) | ones] so the
            # 33-wide per-head lhsT computes o rows 0:32 plus a row-sum row 32
            vt = persist.tile([128, nsc, 2 * (HD + 1)], FP32R, tag="vt")
            nc.sync.dma_start(out=vt[:, :, HD : HD + 1], in_=vtc[:, :, 0:1])
            nc.sync.dma_start(out=vt[:, :, 2 * HD + 1 : 2 * HD + 2], in_=vtc[:, :, 1:2])
            eyeblk = persist.tile([128, nt, nt], FP32R, tag="eyeblk")
            nc.sync.dma_start(out=eyeblk, in_=peye[:, :, :])
            ones1c = persist.tile([1, 128], FP32R, tag="ones1c")
            nc.sync.dma_start(out=ones1c, in_=onesr[:, :])
            selt = persist.tile([nt, nt, 128], FP32R, tag="selt")
            nc.sync.dma_start(out=selt, in_=selp[:, :, :])
            # own heads' normalized attention out, pre-AllGather, [32, t] each
            oTp = [persist.tile([HD, t], FP32R, tag=f"oTp{h}", name=f"oTp{h}") for h in range(HPC)]
            epst = persist.tile([128, 1], FP32, tag="epst")
            nc.vector.memset(epst, EPS)

            def layernorm(src, g_ap_of, b_ap_of, out_tiles):
                if "ln" in ablate:
                    for e in range(ET):
                        nc.scalar.activation(
                            out=out_tiles[e][:, :], in_=src[e][:, :], func=AF.Identity,
                            bias=b_ap_of(e), scale=g_ap_of(e),
                        )
                    return
                """src: list of ET [128, t] fp32r tiles -> out_tiles fp32r.

                Per-token stats via ones-matmuls into PSUM rows {0,32,64,96}
                (one per t-block), then (x*s + m2)*g + b with s=rstd,
                m2=-mean*rstd broadcast along partitions.
                """
                sq = [
                    bigp.tile([128, t], FP32R, tag="big", name=f"sq{e}")
                    for e in range(ET)
                ]
                for e in range(ET):
                    nc.vector.tensor_tensor(
                        out=sq[e], in0=src[e], in1=src[e], op=ALU.mult
                    )
                xs_ps = ps_s.tile([nt, TB], FP32, tag="stat_x")
                qs_ps = ps_s.tile([nt, TB], FP32, tag="stat_q")
                for tb in range(nt):
                    for e in range(ET):
                        nc.tensor.matmul(
                            xs_ps[:, :],
                            eyeblk[:, tb, :],
                            src[e][:, TB * tb : TB * (tb + 1)],
                            start=(tb == 0 and e == 0),
                            stop=(tb == nt - 1 and e == ET - 1),
                        )
                    for e in range(ET):
                        nc.tensor.matmul(
                            qs_ps[:, :],
                            eyeblk[:, tb, :],
                            sq[e][:, TB * tb : TB * (tb + 1)],
                            start=(tb == 0 and e == 0),
                            stop=(tb == nt - 1 and e == ET - 1),
                        )
                mean4 = smallp.tile([nt, TB], FP32, tag="mean4", name="mean4")
                msq4 = smallp.tile([nt, TB], FP32, tag="msq4", name="msq4")
                var4 = smallp.tile([nt, TB], FP32, tag="var4", name="var4")
                s4 = smallp.tile([nt, TB], FP32R, tag="s4", name="s4")
                xs_rows = xs_ps[:, :]
                qs_rows = qs_ps[:, :]
                nc.vector.tensor_scalar(mean4[:, :], xs_rows, 1.0 / E, None, ALU.mult)
                nc.vector.tensor_scalar(msq4[:, :], qs_rows, 1.0 / E, None, ALU.mult)
                nc.vector.tensor_tensor(
                    out=var4[:, :], in0=mean4[:, :], in1=mean4[:, :], op=ALU.mult
                )
                nc.vector.tensor_tensor(
                    out=var4[:, :], in0=msq4[:, :], in1=var4[:, :], op=ALU.subtract
                )
                nc.scalar.activation(out=var4[:, :], in_=var4[:, :], func=AF.Ln, bias=epst[0:nt, :])
                nc.scalar.activation(out=s4[:, :], in_=var4[:, :], func=AF.Exp, scale=-0.5)
                m24 = smallp.tile([nt, TB], FP32R, tag="msq4", name="m24")
                nc.vector.scalar_tensor_tensor(
                    out=m24[:, :], in0=mean4[:, :], scalar=-1.0, in1=s4[:, :],
                    op0=ALU.mult, op1=ALU.mult,
                )
                for tb in range(nt):
                    s_bc = ps_s.tile([128, TB], FP32, tag="stat_x", name="s_bc")
                    m_bc = ps_s.tile([128, TB], FP32, tag="stat_q", name="m_bc")
                    nc.tensor.matmul(
                        s_bc[:, :], selt[:, tb, :], s4[:, :],
                        start=True, stop=True,
                    )
                    nc.tensor.matmul(
                        m_bc[:, :], selt[:, tb, :], m24[:, :],
                        start=True, stop=True,
                    )
                    for e in range(ET):
                        tmp = tmpp.tile([128, TB], FP32, tag="lntmp")
                        nc.vector.tensor_tensor(
                            out=tmp,
                            in0=src[e][:, TB * tb : TB * (tb + 1)],
                            in1=s_bc[:, :], op=ALU.mult,
                        )
                        nc.vector.tensor_tensor(
                            out=tmp, in0=tmp, in1=m_bc[:, :], op=ALU.add,
                        )
                        nc.scalar.activation(
                            out=out_tiles[e][:, TB * tb : TB * (tb + 1)],
                            in_=tmp, func=AF.Identity,
                            bias=b_ap_of(e), scale=g_ap_of(e),
                        )

            # ================= layers =================
            for l in range(layers):
                wq_t = [wpool2.tile([128, 6 * HD], FP32R, tag=f"wqkv{e}", name=f"wqkv{e}") for e in range(ET)]
                wo_t = [wpool2.tile([128, E], FP32R, tag=f"wo{e}", name=f"wot{e}") for e in range(ET)]
                w1_t = [wpool1.tile([128, FF], FP32R, tag=f"w1{e}", name=f"w1t{e}") for e in range(ET)]
                w2_t = wpool1.tile([128, UT, E], FP32R, tag="w2")
                vec = wpool2.tile([128, 20], FP32, tag="vec")
                for e in range(ET):
                    nc.sync.dma_start(out=wq_t[e], in_=wqkv[l, :, e, :])
                    nc.sync.dma_start(out=wo_t[e], in_=wo_p[l, :, e, :])
                    nc.sync.dma_start(out=w1_t[e], in_=w1_p[l, :, e, :])
                nc.sync.dma_start(out=w2_t, in_=w2_p[l, :, :, :])
                nc.sync.dma_start(out=vec, in_=vecs[l, :, :])

                xln = [xlnp.tile([128, t], FP32R, tag=f"xln{e}", name=f"xln{e}") for e in range(ET)]
                layernorm(
                    xT,
                    g_ap_of=lambda e: vec[:, 0 + e : 1 + e],
                    b_ap_of=lambda e: vec[:, 2 + e : 3 + e],
                    out_tiles=xln,
                )

                qT = actp.tile([2 * HD, t], FP32R, tag="qT")
                kT = actp.tile([2 * HD, t], FP32R, tag="kT")
                for tb in range(nt):
                    tsl = slice(TB * tb, TB * (tb + 1))
                    qp = ps_m.tile([2 * HD, TB], FP32, tag="m")
                    for e in range(ET):
                        nc.tensor.matmul(
                            qp[:, :], wq_t[e][:, 0 : 2 * HD], xln[e][:, tsl],
                            start=(e == 0), stop=(e == ET - 1),
                        )
                    nc.any.tensor_copy(out=qT[:, tsl], in_=qp[:, :])
                    kp = ps_m.tile([2 * HD, TB], FP32, tag="m")
                    for e in range(ET):
                        nc.tensor.matmul(
                            kp[:, :], wq_t[e][:, 2 * HD : 4 * HD], xln[e][:, tsl],
                            start=(e == 0), stop=(e == ET - 1),
                        )
                    nc.any.tensor_copy(out=kT[:, tsl], in_=kp[:, :])
                    for i in range(4 * tb, 4 * tb + 4):
                        vp = ps_m.tile([128, 2 * HD], FP32, tag="m")
                        for e in range(ET):
                            nc.tensor.matmul(
                                vp[:, :],
                                xln[e][:, SC * i : SC * (i + 1)],
                                wq_t[e][:, 4 * HD : 6 * HD],
                                start=(e == 0), stop=(e == ET - 1),
                            )
                        nc.any.tensor_copy(out=vt[:, i, 0:HD], in_=vp[:, 0:HD])
                        nc.any.tensor_copy(
                            out=vt[:, i, HD + 1 : 2 * HD + 1], in_=vp[:, HD : 2 * HD]
                        )

                    # ---- attention for this t-block (qkv ready up to here) ----
                    if "attn" in ablate:
                        if tb == 0:
                            for h in range(HPC):
                                nc.vector.memset(oTp[h].bitcast(FP32), 1.0)
                        continue
                    op_ps = [
                        ps_o.tile([HD + 1, TB], FP32, tag=f"o{h}", name=f"op_ps{h}")
                        for h in range(HPC)
                    ]
                    nmax = 4 * tb + 4
                    for h in range(HPC):
                        rsl = slice(32 * h, 32 * (h + 1))

                        def emit_o(i, exh, d):
                            nc.tensor.matmul(
                                op_ps[h][:, d:TB],
                                vt[:, i, (HD + 1) * h : (HD + 1) * h + HD + 1],
                                exh[:, d:TB],
                                start=(i == 0), stop=(i == nmax - 1),
                            )

                        pend = None
                        for i in range(nmax):
                            d = max(0, SC * i - TB * tb)
                            psl = slice(d, TB)
                            tgl = slice(TB * tb + d, TB * (tb + 1))
                            at_ps = ps_a.tile([128, TB], FP32, tag="att", name="at_ps")
                            exh = expp.tile([128, TB], FP32R, tag=f"exp{h}", name="exh")
                            nc.tensor.matmul(
                                at_ps[:, psl],
                                kT[rsl, SC * i : SC * (i + 1)],
                                qT[rsl, tgl],
                                start=True, stop=True,
                                tile_position=(32 * h, 0),
                            )
                            nc.scalar.activation(
                                out=exh[:, psl], in_=at_ps[:, psl],
                                func=AF.Exp, scale=float(E) ** -0.5,
                            )
                            if i >= 4 * tb:  # diagonal chunk: mask upper triangle
                                nc.vector.tensor_tensor(
                                    out=exh[:, d : d + SC],
                                    in0=exh[:, d : d + SC],
                                    in1=mask[:, :], op=ALU.mult,
                                )
                            if pend is not None:
                                emit_o(*pend)
                            pend = (i, exh, d)
                        emit_o(*pend)
                    # normalize each head by its row-sum (psum row 32)
                    srow = [
                        smallp.tile([HD + 1, TB], FP32, tag=f"srow{h}", name=f"srow{h}")
                        for h in range(HPC)
                    ]
                    rd = dpool.tile([HPC, TB], FP32, tag="rd", name="rd")
                    for h in range(HPC):
                        nc.vector.reciprocal(
                            out=srow[h][HD : HD + 1, :],
                            in_=op_ps[h][HD : HD + 1, :],
                        )
                        nc.sync.dma_start(
                            out=rd[h : h + 1, :],
                            in_=srow[h][HD : HD + 1, :],
                        )
                    rec_bc = tmpp.tile([HD, TB], FP32, tag="rbc", name="rec_bc")
                    for h in range(HPC):
                        nc.gpsimd.dma_start(
                            out=rec_bc,
                            in_=rd[h : h + 1, :].partition_broadcast(HD),
                        )
                        nc.vector.tensor_tensor(
                            out=oTp[h][:, TB * tb : TB * (tb + 1)],
                            in0=op_ps[h][0:HD, :],
                            in1=rec_bc,
                            op=ALU.mult,
                        )

                # ---- AllGather heads across the 4-core batch group ----
                oT = [actp.tile([128, t], FP32R, tag=tg, name=f"oT_{tg}") for tg in ("qT", "kT")]
                if use_collective:
                    for h in range(HPC):
                        nc.sync.dma_start(
                            out=cc_in[l][HD * h : HD * (h + 1), :], in_=oTp[h][:, :]
                        )
                    nc.gpsimd.collective_compute(
                        "AllGather", ALU.bypass,
                        replica_groups=groups,
                        ins=[cc_in[l][:, :]], outs=[cc_out[l][:, :]],
                    )
                    for e in range(ET):
                        nc.sync.dma_start(
                            out=oT[e], in_=cc_out[l][128 * e : 128 * (e + 1), :]
                        )
                else:  # single-group debug path (no comm): own heads only
                    for e in range(ET):
                        nc.vector.memset(oT[e].bitcast(FP32), 0.0)
                    for h in range(HPC):
                        nc.vector.tensor_copy(
                            out=oT[0][HD * h : HD * (h + 1), :].bitcast(FP32),
                            in_=oTp[h][:, :].bitcast(FP32),
                        )

                # ---- wo projection + residual ----
                for tb in range(nt):
                    tsl = slice(TB * tb, TB * (tb + 1))
                    for eo in range(ET):
                        wp = ps_m.tile([128, TB], FP32, tag="m")
                        for e in range(ET):
                            nc.tensor.matmul(
                                wp[:, :],
                                wo_t[e][:, 128 * eo : 128 * (eo + 1)],
                                oT[e][:, tsl],
                                start=(e == 0), stop=(e == ET - 1),
                            )
                        nc.vector.scalar_tensor_tensor(
                            out=xT[eo][:, tsl], in0=wp[:, :],
                            scalar=vec[:, 8 + eo : 9 + eo], in1=xT[eo][:, tsl],
                            op0=ALU.add, op1=ALU.add,
                        )

                # ---- FFN ----
                xln2 = [xlnp.tile([128, t], FP32R, tag=f"xln{e}", name=f"xln{e}") for e in range(ET)]
                layernorm(
                    xT,
                    g_ap_of=lambda e: vec[:, 4 + e : 5 + e],
                    b_ap_of=lambda e: vec[:, 6 + e : 7 + e],
                    out_tiles=xln2,
                )
                for tb in range(nt):
                    if "ffn" in ablate:
                        break
                    tsl = slice(TB * tb, TB * (tb + 1))
                    ru_halves = []
                    for half in range(2):
                        ru = bigp.tile([128, UT // 2, TB], FP32R, tag="big", name="ru")
                        for uu in range(UT // 2):
                            ut = half * (UT // 2) + uu
                            up = ps_a.tile([128, TB], FP32, tag="att", name="up")
                            for e in range(ET):
                                nc.tensor.matmul(
                                    up[:, :],
                                    w1_t[e][:, 128 * ut : 128 * (ut + 1)],
                                    xln2[e][:, tsl],
                                    start=(e == 0), stop=(e == ET - 1),
                                )
                            nc.scalar.activation(
                                out=ru[:, uu, :], in_=up[:, :], func=AF.Relu,
                                bias=vec[:, 10 + ut : 11 + ut],
                            )
                        ru_halves.append(ru)
                    for eo in range(ET):
                        wp2 = ps_m.tile([128, TB], FP32, tag="m", name="wp2")
                        for ut in range(UT):
                            nc.tensor.matmul(
                                wp2[:, :],
                                w2_t[:, ut, 128 * eo : 128 * (eo + 1)],
                                ru_halves[ut // (UT // 2)][:, ut % (UT // 2), :],
                                start=(ut == 0), stop=(ut == UT - 1),
                            )
                        nc.vector.scalar_tensor_tensor(
                            out=xT[eo][:, tsl], in0=wp2[:, :],
                            scalar=vec[:, 18 + eo : 19 + eo], in1=xT[eo][:, tsl],
                            op0=ALU.add, op1=ALU.add,
                        )

            # ================= final LN + lm_head =================
            xf = [xlnp.tile([128, t], FP32R, tag=f"xln{e}", name=f"xln{e}") for e in range(ET)]
            layernorm(
                xT,
                g_ap_of=lambda e: fv[:, 0 + e : 1 + e],
                b_ap_of=lambda e: fv[:, 2 + e : 3 + e],
                out_tiles=xf,
            )
            for vb in range(nvb if "lm" not in ablate else 1):
                wh = whp.tile([128, ET, 512], FP32R, tag="wh")
                nc.sync.dma_start(out=wh, in_=whead[:, :, 512 * vb : 512 * (vb + 1)])
                for tcn in range(ntc):
                    lp = ps_m.tile([128, 512], FP32, tag="m")
                    for e in range(ET):
                        nc.tensor.matmul(
                            lp[:, :],
                            xf[e][:, 128 * tcn : 128 * (tcn + 1)],
                            wh[:, e, :],
                            start=(e == 0), stop=(e == ET - 1),
                        )
                    lg = lgp.tile([128, 512], FP32, tag="lg")
                    if (vb + tcn) % 2 == 0:
                        nc.vector.tensor_copy(out=lg, in_=lp[:, :])
                    else:
                        nc.scalar.copy(out=lg, in_=lp[:, :])
                    nc.sync.dma_start(
                        out=logits[128 * tcn : 128 * (tcn + 1), 512 * vb : 512 * (vb + 1)],
                        in_=lg,
                    )

    nc.compile()
    return nc


# ---------------- host-side prep / unshard ----------------

def prep_core_inputs(c, X, tok_emb, pos_emb, wq, wk, wv, wo, bo, w1, b1, w2, b2,
                     ln1_g, ln1_b, ln2_g, ln2_b, lnf_g, lnf_b, w_head, b_head,
                     t=T, layers=L, vsp=VSP):
    b = c // GROUP
    j = c % GROUP
    heads = [HPC * j + k for k in range(HPC)]

    f32 = np.float32
    Xb = np.asarray(X[b]).astype(np.int64)
    x0 = (np.asarray(tok_emb)[Xb] + np.asarray(pos_emb)[:t]).astype(f32).T  # [E, t]

    wq = np.asarray(wq); wk = np.asarray(wk); wv = np.asarray(wv)
    wqkv_h = np.empty((layers, 128, ET, 6 * HD), f32)
    wo_h = np.empty((layers, 128, ET, E), f32)
    w1_h = np.empty((layers, 128, ET, FF), f32)
    w2_h = np.empty((layers, 128, UT, E), f32)
    vecs_h = np.empty((layers, 128, 20), f32)
    for l in range(layers):
        qc = np.concatenate([wq[l, h] for h in heads], axis=1)  # [E, 64]
        kc = np.concatenate([wk[l, h] for h in heads], axis=1)
        vc = np.concatenate([wv[l, h] for h in heads], axis=1)
        qkv = np.concatenate([qc, kc, vc], axis=1)  # [E, 192]
        wqkv_h[l] = qkv.reshape(ET, 128, 6 * HD).transpose(1, 0, 2)
        wo_h[l] = np.asarray(wo[l]).reshape(ET, 128, E).transpose(1, 0, 2)
        w1_h[l] = np.asarray(w1[l]).reshape(ET, 128, FF).transpose(1, 0, 2)
        w2_h[l] = np.asarray(w2[l]).reshape(UT, 128, E).transpose(1, 0, 2)
        vv = np.concatenate([
            np.asarray(ln1_g[l]), np.asarray(ln1_b[l]),
            np.asarray(ln2_g[l]), np.asarray(ln2_b[l]),
            np.asarray(bo[l]), np.asarray(b1[l]), np.asarray(b2[l]),
        ]).astype(f32)  # 2560
        vecs_h[l] = vv.reshape(20, 128).T
    fvec_h = np.concatenate(
        [np.asarray(lnf_g), np.asarray(lnf_b)]
    ).astype(f32).reshape(4, 128).T

    w_head = np.asarray(w_head)
    vs = w_head.shape[1] // GROUP
    wh = np.zeros((E, vsp), f32)
    wh[:, :vs] = w_head[:, vs * j : vs * (j + 1)]
    whead_h = np.ascontiguousarray(wh.reshape(ET, 128, vsp).transpose(1, 0, 2))

    sp = np.arange(SC)[:, None]
    tp = np.arange(SC)[None, :]
    mask_h = (sp <= tp).astype(f32)

    nt = t // TB
    nsc = t // SC
    peye_h = np.zeros((128, nt, nt), f32)
    for tb in range(nt):
        peye_h[:, tb, tb] = 1.0
    vtc_h = np.ones((128, nsc, 2), f32)

    return {
        "x0": np.ascontiguousarray(x0),
        "wqkv": np.ascontiguousarray(wqkv_h),
        "wo": np.ascontiguousarray(wo_h),
        "w1": np.ascontiguousarray(w1_h),
        "w2": np.ascontiguousarray(w2_h),
        "vecs": np.ascontiguousarray(vecs_h),
        "fvec": np.ascontiguousarray(fvec_h),
        "whead": whead_h,
        "mask": mask_h,
        "peye": peye_h,
        "vtc": vtc_h,
        "onesr": np.ones((1, 128), f32),
        "selp": np.ascontiguousarray(
            np.broadcast_to(np.eye(nt, dtype=f32)[:, :, None], (nt, nt, 128))
        ),
    }


_NC_CACHE = {}


def _get_nc():
    if "nc" not in _NC_CACHE:
        _NC_CACHE["nc"] = build_nc()
    return _NC_CACHE["nc"]


def kernel(**inputs):
    nc = _get_nc()
    in_maps = [prep_core_inputs(c, **inputs) for c in range(NCORES)]
    res = run_bass_kernel_spmd(nc, in_maps, list(range(NCORES)))
    out = np.empty((B, T, V), np.float32)
    for c in range(NCORES):
        b, j = c // GROUP, c % GROUP
        out[b, :, VS * j : VS * (j + 1)] = res.results[c]["logits"][:, :VS]
    b_head = np.asarray(inputs["b_head"])
    if np.any(b_head):
        out += b_head[None, None, :]
    return out


# revision 29
# speedup vs baseline: 30455.2926x; 1.0001x over previous
"""Trainium2 Bass kernel for a 4-layer bigram-LM dense transformer.

Full-model shapes: B=2, T=2048, E=256, H=8, L=4, V=32000.

Sharding over 8 NeuronCores (self-contained, hardcoded):
  - 2-way data parallel over batch: cores 0-3 handle batch 0, cores 4-7
    batch 1 (a "batch group" of 4 cores each).
  - Within a batch group, per-token work (LN / QKV / wo / FFN) is
    replicated; attention (the exp-heavy part) is sharded 2 heads/core
    and re-assembled with one 4-rank AllGather per layer.
  - lm_head is sharded 4-way over vocab columns within the group
    (8000 cols/core, padded to 8192), so the dominant 524MB logits
    write is split 8 ways.

Compute layout: activations live transposed [E, T] in SBUF so every
matmul contracts over the partition axis with zero transposes. All
matmul operands are float32r (FP22 mantissa truncation, full PE rate
at moving-dim >= 256). Softmax skips the max-subtraction (scores are
~1e-1 scale; exp cannot overflow) and row sums ride along in the
attention-output matmul via a ones column packed next to V, with
normalization folded into the PSUM->SBUF copy.
"""

import numpy as np

import concourse.bass as bass
import concourse.mybir as mybir
import concourse.tile as tile
from concourse import bacc
from concourse.bass_utils import run_bass_kernel_spmd

AF = mybir.ActivationFunctionType
ALU = mybir.AluOpType
FP32 = mybir.dt.float32
FP32R = mybir.dt.float32r

# model dims (full problem)
B, T, E, H, L, V = 2, 2048, 256, 8, 4, 32000
HD = E // H  # 32
EPS = 1e-5
NCORES = 8
GROUP = 4  # cores per batch group
HPC = H // GROUP  # heads per core (2)
VS = V // GROUP  # vocab shard per core (8000)
VSP = 8192  # padded vocab shard
TB = 512  # t-block (PSUM bank free dim)
SC = 128  # s-chunk (partition dim)
ET = E // 128  # embedding partition tiles (2)
FF = 4 * E  # 1024
UT = FF // 128  # ffn u-tiles (8)


def build_nc(t=T, layers=L, vsp=VSP, use_collective=True, ablate=()):
    """Build + compile the per-core Bass program (SPMD: same program, 8 cores)."""
    nt = t // TB      # t-blocks
    nsc = t // SC     # s-chunks
    ntc = t // 128    # t-chunks for lm head
    nvb = vsp // 512  # vocab blocks

    nc = bacc.Bacc("TRN2", num_devices=NCORES)

    # ---- DRAM parameters (per core) ----
    x0 = nc.declare_dram_parameter("x0", [E, t], FP32R, isOutput=False)
    wqkv = nc.declare_dram_parameter("wqkv", [layers, 128, ET, 6 * HD], FP32R, isOutput=False)
    wo_p = nc.declare_dram_parameter("wo", [layers, 128, ET, E], FP32R, isOutput=False)
    w1_p = nc.declare_dram_parameter("w1", [layers, 128, ET, FF], FP32R, isOutput=False)
    w2_p = nc.declare_dram_parameter("w2", [layers, 128, UT, E], FP32R, isOutput=False)
    vecs = nc.declare_dram_parameter("vecs", [layers, 128, 20], FP32, isOutput=False)
    fvec = nc.declare_dram_parameter("fvec", [128, 4], FP32, isOutput=False)
    whead = nc.declare_dram_parameter("whead", [128, ET, vsp], FP32R, isOutput=False)
    maskp = nc.declare_dram_parameter("mask", [SC, SC], FP32, isOutput=False)
    peye = nc.declare_dram_parameter("peye", [128, nt, nt], FP32R, isOutput=False)
    vtc = nc.declare_dram_parameter("vtc", [128, nsc, 2], FP32R, False)
    onesr = nc.declare_dram_parameter("onesr", [1, 128], FP32R, isOutput=False)
    selp = nc.declare_dram_parameter("selp", [nt, nt, 128], FP32R, isOutput=False)
    onesc = nc.declare_dram_parameter("onesc", [HD + 1, 128], FP32R, isOutput=False)
    logits = nc.declare_dram_parameter("logits", [t, vsp], FP32, isOutput=True)

    # internal DRAM bounce buffers for the per-layer AllGather
    cc_in = [nc.dram_tensor(f"cc_in{l}", [HPC * HD, t], FP32R) for l in range(layers)]
    cc_out = [nc.dram_tensor(f"cc_out{l}", [GROUP * HPC * HD, t], FP32R) for l in range(layers)]
    groups = [[0, 1, 2, 3], [4, 5, 6, 7]]

    from contextlib import ExitStack
    with tile.TileContext(nc) as tc:
        with ExitStack() as _ctx:
            persist = _ctx.enter_context(tc.tile_pool(name="persist", bufs=1))
            wpool2 = _ctx.enter_context(tc.tile_pool(name="wpool2", bufs=2))
            wpool1 = _ctx.enter_context(tc.tile_pool(name="wpool1", bufs=1))
            actp = _ctx.enter_context(tc.tile_pool(name="actp", bufs=1))
            xlnp = _ctx.enter_context(tc.tile_pool(name="xlnp", bufs=2))
            bigp = _ctx.enter_context(tc.tile_pool(name="bigp", bufs=3))
            expp = _ctx.enter_context(tc.tile_pool(name="expp", bufs=3))
            smallp = _ctx.enter_context(tc.tile_pool(name="smallp", bufs=2))
            tmpp = _ctx.enter_context(tc.tile_pool(name="tmpp", bufs=2))
            lgp = _ctx.enter_context(tc.tile_pool(name="lgp", bufs=6))
            whp = _ctx.enter_context(tc.tile_pool(name="whp", bufs=2))
            dpool = _ctx.enter_context(tc.tile_pool(name="dpool", bufs=2, space="DRAM"))
            ps_a = _ctx.enter_context(tc.tile_pool(name="ps_a", bufs=2, space="PSUM"))
            ps_o = _ctx.enter_context(tc.tile_pool(name="ps_o", bufs=1, space="PSUM"))
            ps_m = _ctx.enter_context(tc.tile_pool(name="ps_m", bufs=2, space="PSUM"))
            ps_s = _ctx.enter_context(tc.tile_pool(name="ps_s", bufs=1, space="PSUM"))
            # ---- persistent tiles ----
            xT = [persist.tile([128, t], FP32R, tag=f"xT{e}", name=f"xT{e}") for e in range(ET)]
            for e in range(ET):
                nc.sync.dma_start(out=xT[e], in_=x0[128 * e : 128 * (e + 1), :])
            mask = persist.tile([SC, SC], FP32, tag="mask")
            nc.sync.dma_start(out=mask, in_=maskp[:, :])
            fv = persist.tile([128, 4], FP32, tag="fvec")
            nc.sync.dma_start(out=fv, in_=fvec[:, :])
            # v tile: per chunk cols = [vA(32) | ones | vB(32) | ones] so the
            # 33-wide per-head lhsT computes o rows 0:32 plus a row-sum row 32
            vt = persist.tile([128, nsc, 2 * (HD + 1)], FP32R, tag="vt")
            nc.sync.dma_start(out=vt[:, :, HD : HD + 1], in_=vtc[:, :, 0:1])
            nc.sync.dma_start(out=vt[:, :, 2 * HD + 1 : 2 * HD + 2], in_=vtc[:, :, 1:2])
            eyeblk = persist.tile([128, nt, nt], FP32R, tag="eyeblk")
            nc.sync.dma_start(out=eyeblk, in_=peye[:, :, :])
            ones1c = persist.tile([1, 128], FP32R, tag="ones1c")
            nc.sync.dma_start(out=ones1c, in_=onesr[:, :])
            selt = persist.tile([nt, nt, 128], FP32R, tag="selt")
            nc.sync.dma_start(out=selt, in_=selp[:, :, :])
            ones33 = persist.tile([HD + 1, 128], FP32R, tag="ones33")
            nc.sync.dma_start(out=ones33, in_=onesc[:, :])
            # own heads' normalized attention out, pre-AllGather, [32, t] each
            oTp = [persist.tile([HD, t], FP32R, tag=f"oTp{h}", name=f"oTp{h}") for h in range(HPC)]
            epst = persist.tile([128, 1], FP32, tag="epst")
            nc.vector.memset(epst, EPS)

            def layernorm(src, g_ap_of, b_ap_of, out_tiles):
                if "ln" in ablate:
                    for e in range(ET):
                        nc.scalar.activation(
                            out=out_tiles[e][:, :], in_=src[e][:, :], func=AF.Identity,
                            bias=b_ap_of(e), scale=g_ap_of(e),
                        )
                    return
                """src: list of ET [128, t] fp32r tiles -> out_tiles fp32r.

                Per-token stats via ones-matmuls into PSUM rows {0,32,64,96}
                (one per t-block), then (x*s + m2)*g + b with s=rstd,
                m2=-mean*rstd broadcast along partitions.
                """
                sq = [
                    bigp.tile([128, t], FP32R, tag="big", name=f"sq{e}")
                    for e in range(ET)
                ]
                for e in range(ET):
                    nc.vector.tensor_tensor(
                        out=sq[e], in0=src[e], in1=src[e], op=ALU.mult
                    )
                xs_ps = ps_s.tile([nt, TB], FP32, tag="stat_x")
                qs_ps = ps_s.tile([nt, TB], FP32, tag="stat_q")
                for tb in range(nt):
                    for e in range(ET):
                        nc.tensor.matmul(
                            xs_ps[:, :],
                            eyeblk[:, tb, :],
                            src[e][:, TB * tb : TB * (tb + 1)],
                            start=(tb == 0 and e == 0),
                            stop=(tb == nt - 1 and e == ET - 1),
                        )
                    for e in range(ET):
                        nc.tensor.matmul(
                            qs_ps[:, :],
                            eyeblk[:, tb, :],
                            sq[e][:, TB * tb : TB * (tb + 1)],
                            start=(tb == 0 and e == 0),
                            stop=(tb == nt - 1 and e == ET - 1),
                        )
                mean4 = smallp.tile([nt, TB], FP32, tag="mean4", name="mean4")
                msq4 = smallp.tile([nt, TB], FP32, tag="msq4", name="msq4")
                var4 = smallp.tile([nt, TB], FP32, tag="var4", name="var4")
                s4 = smallp.tile([nt, TB], FP32R, tag="s4", name="s4")
                xs_rows = xs_ps[:, :]
                qs_rows = qs_ps[:, :]
                nc.vector.tensor_scalar(mean4[:, :], xs_rows, 1.0 / E, None, ALU.mult)
                nc.vector.tensor_scalar(msq4[:, :], qs_rows, 1.0 / E, None, ALU.mult)
                nc.vector.tensor_tensor(
                    out=var4[:, :], in0=mean4[:, :], in1=mean4[:, :], op=ALU.mult
                )
                nc.vector.tensor_tensor(
                    out=var4[:, :], in0=msq4[:, :], in1=var4[:, :], op=ALU.subtract
                )
                nc.scalar.activation(out=var4[:, :], in_=var4[:, :], func=AF.Ln, bias=epst[0:nt, :])
                nc.scalar.activation(out=s4[:, :], in_=var4[:, :], func=AF.Exp, scale=-0.5)
                m24 = smallp.tile([nt, TB], FP32R, tag="msq4", name="m24")
                nc.vector.scalar_tensor_tensor(
                    out=m24[:, :], in0=mean4[:, :], scalar=-1.0, in1=s4[:, :],
                    op0=ALU.mult, op1=ALU.mult,
                )
                for tb in range(nt):
                    s_bc = ps_s.tile([128, TB], FP32, tag="stat_x", name="s_bc")
                    m_bc = ps_s.tile([128, TB], FP32, tag="stat_q", name="m_bc")
                    nc.tensor.matmul(
                        s_bc[:, :], selt[:, tb, :], s4[:, :],
                        start=True, stop=True,
                    )
                    nc.tensor.matmul(
                        m_bc[:, :], selt[:, tb, :], m24[:, :],
                        start=True, stop=True,
                    )
                    for e in range(ET):
                        tmp = tmpp.tile([128, TB], FP32, tag="lntmp")
                        nc.vector.tensor_tensor(
                            out=tmp,
                            in0=src[e][:, TB * tb : TB * (tb + 1)],
                            in1=s_bc[:, :], op=ALU.mult,
                        )
                        nc.vector.tensor_tensor(
                            out=tmp, in0=tmp, in1=m_bc[:, :], op=ALU.add,
                        )
                        nc.scalar.activation(
                            out=out_tiles[e][:, TB * tb : TB * (tb + 1)],
                            in_=tmp, func=AF.Identity,
                            bias=b_ap_of(e), scale=g_ap_of(e),
                        )

            # ================= layers =================
            for l in range(layers):
                wq_t = [wpool2.tile([128, 6 * HD], FP32R, tag=f"wqkv{e}", name=f"wqkv{e}") for e in range(ET)]
                wo_t = [wpool2.tile([128, E], FP32R, tag=f"wo{e}", name=f"wot{e}") for e in range(ET)]
                w1_t = [wpool1.tile([128, FF], FP32R, tag=f"w1{e}", name=f"w1t{e}") for e in range(ET)]
                w2_t = wpool1.tile([128, UT, E], FP32R, tag="w2")
                vec = wpool2.tile([128, 20], FP32, tag="vec")
                for e in range(ET):
                    nc.sync.dma_start(out=wq_t[e], in_=wqkv[l, :, e, :])
                    nc.sync.dma_start(out=wo_t[e], in_=wo_p[l, :, e, :])
                    nc.sync.dma_start(out=w1_t[e], in_=w1_p[l, :, e, :])
                nc.sync.dma_start(out=w2_t, in_=w2_p[l, :, :, :])
                nc.sync.dma_start(out=vec, in_=vecs[l, :, :])

                xln = [xlnp.tile([128, t], FP32R, tag=f"xln{e}", name=f"xln{e}") for e in range(ET)]
                layernorm(
                    xT,
                    g_ap_of=lambda e: vec[:, 0 + e : 1 + e],
                    b_ap_of=lambda e: vec[:, 2 + e : 3 + e],
                    out_tiles=xln,
                )

                qT = actp.tile([2 * HD, t], FP32R, tag="qT")
                kT = actp.tile([2 * HD, t], FP32R, tag="kT")
                for tb in range(nt):
                    tsl = slice(TB * tb, TB * (tb + 1))
                    qp = ps_m.tile([2 * HD, TB], FP32, tag="m")
                    for e in range(ET):
                        nc.tensor.matmul(
                            qp[:, :], wq_t[e][:, 0 : 2 * HD], xln[e][:, tsl],
                            start=(e == 0), stop=(e == ET - 1),
                        )
                    nc.any.tensor_copy(out=qT[:, tsl], in_=qp[:, :])
                    kp = ps_m.tile([2 * HD, TB], FP32, tag="m")
                    for e in range(ET):
                        nc.tensor.matmul(
                            kp[:, :], wq_t[e][:, 2 * HD : 4 * HD], xln[e][:, tsl],
                            start=(e == 0), stop=(e == ET - 1),
                        )
                    nc.any.tensor_copy(out=kT[:, tsl], in_=kp[:, :])
                    for i in range(4 * tb, 4 * tb + 4):
                        vp = ps_m.tile([128, 2 * HD], FP32, tag="m")
                        for e in range(ET):
                            nc.tensor.matmul(
                                vp[:, :],
                                xln[e][:, SC * i : SC * (i + 1)],
                                wq_t[e][:, 4 * HD : 6 * HD],
                                start=(e == 0), stop=(e == ET - 1),
                            )
                        nc.any.tensor_copy(out=vt[:, i, 0:HD], in_=vp[:, 0:HD])
                        nc.any.tensor_copy(
                            out=vt[:, i, HD + 1 : 2 * HD + 1], in_=vp[:, HD : 2 * HD]
                        )

                    # ---- attention for this t-block (qkv ready up to here) ----
                    if "attn" in ablate:
                        if tb == 0:
                            for h in range(HPC):
                                nc.vector.memset(oTp[h].bitcast(FP32), 1.0)
                        continue
                    op_ps = [
                        ps_o.tile([HD + 1, TB], FP32, tag=f"o{h}", name=f"op_ps{h}")
                        for h in range(HPC)
                    ]
                    nmax = 4 * tb + 4
                    for h in range(HPC):
                        rsl = slice(32 * h, 32 * (h + 1))

                        def emit_o(i, exh, d):
                            nc.tensor.matmul(
                                op_ps[h][:, d:TB],
                                vt[:, i, (HD + 1) * h : (HD + 1) * h + HD + 1],
                                exh[:, d:TB],
                                start=(i == 0), stop=(i == nmax - 1),
                            )

                        pend = None
                        for i in range(nmax):
                            d = max(0, SC * i - TB * tb)
                            psl = slice(d, TB)
                            tgl = slice(TB * tb + d, TB * (tb + 1))
                            at_ps = ps_a.tile([128, TB], FP32, tag="att", name="at_ps")
                            exh = expp.tile([128, TB], FP32R, tag=f"exp{h}", name="exh")
                            nc.tensor.matmul(
                                at_ps[:, psl],
                                kT[rsl, SC * i : SC * (i + 1)],
                                qT[rsl, tgl],
                                start=True, stop=True,
                                tile_position=(32 * h, 0),
                            )
                            nc.scalar.activation(
                                out=exh[:, psl], in_=at_ps[:, psl],
                                func=AF.Exp, scale=float(E) ** -0.5,
                            )
                            if i >= 4 * tb:  # diagonal chunk: mask upper triangle
                                nc.vector.tensor_tensor(
                                    out=exh[:, d : d + SC],
                                    in0=exh[:, d : d + SC],
                                    in1=mask[:, :], op=ALU.mult,
                                )
                            if pend is not None:
                                emit_o(*pend)
                            pend = (i, exh, d)
                        emit_o(*pend)
                    # normalize each head by its row-sum (psum row 32)
                    srow = [
                        smallp.tile([HD + 1, TB], FP32, tag=f"srow{h}", name=f"srow{h}")
                        for h in range(HPC)
                    ]
                    rd = dpool.tile([HPC, TB], FP32, tag="rd", name="rd")
                    for h in range(HPC):
                        nc.vector.reciprocal(
                            out=srow[h][HD : HD + 1, :],
                            in_=op_ps[h][HD : HD + 1, :],
                        )
                        nc.sync.dma_start(
                            out=rd[h : h + 1, :],
                            in_=srow[h][HD : HD + 1, :],
                        )
                    rec_bc = tmpp.tile([HD, TB], FP32, tag="rbc", name="rec_bc")
                    for h in range(HPC):
                        nc.gpsimd.dma_start(
                            out=rec_bc,
                            in_=rd[h : h + 1, :].partition_broadcast(HD),
                        )
                        nc.vector.tensor_tensor(
                            out=oTp[h][:, TB * tb : TB * (tb + 1)],
                            in0=op_ps[h][0:HD, :],
                            in1=rec_bc,
                            op=ALU.mult,
                        )

                # ---- AllGather heads across the 4-core batch group ----
                oT = [actp.tile([128, t], FP32R, tag=tg, name=f"oT_{tg}") for tg in ("qT", "kT")]
                if use_collective:
                    for h in range(HPC):
                        nc.sync.dma_start(
                            out=cc_in[l][HD * h : HD * (h + 1), :], in_=oTp[h][:, :]
                        )
                    nc.gpsimd.collective_compute(
                        "AllGather", ALU.bypass,
                        replica_groups=groups,
                        ins=[cc_in[l][:, :]], outs=[cc_out[l][:, :]],
                    )
                    for e in range(ET):
                        nc.sync.dma_start(
                            out=oT[e], in_=cc_out[l][128 * e : 128 * (e + 1), :]
                        )
                else:  # single-group debug path (no comm): own heads only
                    for e in range(ET):
                        nc.vector.memset(oT[e].bitcast(FP32), 0.0)
                    for h in range(HPC):
                        nc.vector.tensor_copy(
                            out=oT[0][HD * h : HD * (h + 1), :].bitcast(FP32),
                            in_=oTp[h][:, :].bitcast(FP32),
                        )

                # ---- wo projection + residual ----
                for tb in range(nt):
                    tsl = slice(TB * tb, TB * (tb + 1))
                    for eo in range(ET):
                        wp = ps_m.tile([128, TB], FP32, tag="m")
                        for e in range(ET):
                            nc.tensor.matmul(
                                wp[:, :],
                                wo_t[e][:, 128 * eo : 128 * (eo + 1)],
                                oT[e][:, tsl],
                                start=(e == 0), stop=(e == ET - 1),
                            )
                        nc.vector.scalar_tensor_tensor(
                            out=xT[eo][:, tsl], in0=wp[:, :],
                            scalar=vec[:, 8 + eo : 9 + eo], in1=xT[eo][:, tsl],
                            op0=ALU.add, op1=ALU.add,
                        )

                # ---- FFN ----
                xln2 = [xlnp.tile([128, t], FP32R, tag=f"xln{e}", name=f"xln{e}") for e in range(ET)]
                layernorm(
                    xT,
                    g_ap_of=lambda e: vec[:, 4 + e : 5 + e],
                    b_ap_of=lambda e: vec[:, 6 + e : 7 + e],
                    out_tiles=xln2,
                )
                for tb in range(nt):
                    if "ffn" in ablate:
                        break
                    tsl = slice(TB * tb, TB * (tb + 1))
                    ru_halves = []
                    for half in range(2):
                        ru = bigp.tile([128, UT // 2, TB], FP32R, tag="big", name="ru")
                        for uu in range(UT // 2):
                            ut = half * (UT // 2) + uu
                            up = ps_a.tile([128, TB], FP32, tag="att", name="up")
                            for e in range(ET):
                                nc.tensor.matmul(
                                    up[:, :],
                                    w1_t[e][:, 128 * ut : 128 * (ut + 1)],
                                    xln2[e][:, tsl],
                                    start=(e == 0), stop=(e == ET - 1),
                                )
                            nc.scalar.activation(
                                out=ru[:, uu, :], in_=up[:, :], func=AF.Relu,
                                bias=vec[:, 10 + ut : 11 + ut],
                            )
                        ru_halves.append(ru)
                    for eo in range(ET):
                        wp2 = ps_m.tile([128, TB], FP32, tag="m", name="wp2")
                        for ut in range(UT):
                            nc.tensor.matmul(
                                wp2[:, :],
                                w2_t[:, ut, 128 * eo : 128 * (eo + 1)],
                                ru_halves[ut // (UT // 2)][:, ut % (UT // 2), :],
                                start=(ut == 0), stop=(ut == UT - 1),
                            )
                        nc.vector.scalar_tensor_tensor(
                            out=xT[eo][:, tsl], in0=wp2[:, :],
                            scalar=vec[:, 18 + eo : 19 + eo], in1=xT[eo][:, tsl],
                            op0=ALU.add, op1=ALU.add,
                        )

            # ================= final LN + lm_head =================
            xf = [xlnp.tile([128, t], FP32R, tag=f"xln{e}", name=f"xln{e}") for e in range(ET)]
            layernorm(
                xT,
                g_ap_of=lambda e: fv[:, 0 + e : 1 + e],
                b_ap_of=lambda e: fv[:, 2 + e : 3 + e],
                out_tiles=xf,
            )
            for vb in range(nvb if "lm" not in ablate else 1):
                wh = whp.tile([128, ET, 512], FP32R, tag="wh")
                nc.sync.dma_start(out=wh, in_=whead[:, :, 512 * vb : 512 * (vb + 1)])
                for tcn in range(ntc):
                    lp = ps_m.tile([128, 512], FP32, tag="m")
                    for e in range(ET):
                        nc.tensor.matmul(
                            lp[:, :],
                            xf[e][:, 128 * tcn : 128 * (tcn + 1)],
                            wh[:, e, :],
                            start=(e == 0), stop=(e == ET - 1),
                        )
                    lg = lgp.tile([128, 512], FP32, tag="lg")
                    if (vb + tcn) % 2 == 0:
                        nc.vector.tensor_copy(out=lg, in_=lp[:, :])
                    else:
                        nc.scalar.copy(out=lg, in_=lp[:, :])
                    nc.sync.dma_start(
                        out=logits[128 * tcn : 128 * (tcn + 1), 512 * vb : 512 * (vb + 1)],
                        in_=lg,
                    )

    nc.compile()
    return nc


# ---------------- host-side prep / unshard ----------------

def prep_core_inputs(c, X, tok_emb, pos_emb, wq, wk, wv, wo, bo, w1, b1, w2, b2,
                     ln1_g, ln1_b, ln2_g, ln2_b, lnf_g, lnf_b, w_head, b_head,
                     t=T, layers=L, vsp=VSP):
    b = c // GROUP
    j = c % GROUP
    heads = [HPC * j + k for k in range(HPC)]

    f32 = np.float32
    Xb = np.asarray(X[b]).astype(np.int64)
    x0 = (np.asarray(tok_emb)[Xb] + np.asarray(pos_emb)[:t]).astype(f32).T  # [E, t]

    wq = np.asarray(wq); wk = np.asarray(wk); wv = np.asarray(wv)
    wqkv_h = np.empty((layers, 128, ET, 6 * HD), f32)
    wo_h = np.empty((layers, 128, ET, E), f32)
    w1_h = np.empty((layers, 128, ET, FF), f32)
    w2_h = np.empty((layers, 128, UT, E), f32)
    vecs_h = np.empty((layers, 128, 20), f32)
    for l in range(layers):
        qc = np.concatenate([wq[l, h] for h in heads], axis=1)  # [E, 64]
        kc = np.concatenate([wk[l, h] for h in heads], axis=1)
        vc = np.concatenate([wv[l, h] for h in heads], axis=1)
        qkv = np.concatenate([qc, kc, vc], axis=1)  # [E, 192]
        wqkv_h[l] = qkv.reshape(ET, 128, 6 * HD).transpose(1, 0, 2)
        wo_h[l] = np.asarray(wo[l]).reshape(ET, 128, E).transpose(1, 0, 2)
        w1_h[l] = np.asarray(w1[l]).reshape(ET, 128, FF).transpose(1, 0, 2)
        w2_h[l] = np.asarray(w2[l]).reshape(UT, 128, E).transpose(1, 0, 2)
        vv = np.concatenate([
            np.asarray(ln1_g[l]), np.asarray(ln1_b[l]),
            np.asarray(ln2_g[l]), np.asarray(ln2_b[l]),
            np.asarray(bo[l]), np.asarray(b1[l]), np.asarray(b2[l]),
        ]).astype(f32)  # 2560
        vecs_h[l] = vv.reshape(20, 128).T
    fvec_h = np.concatenate(
        [np.asarray(lnf_g), np.asarray(lnf_b)]
    ).astype(f32).reshape(4, 128).T

    w_head = np.asarray(w_head)
    vs = w_head.shape[1] // GROUP
    wh = np.zeros((E, vsp), f32)
    wh[:, :vs] = w_head[:, vs * j : vs * (j + 1)]
    whead_h = np.ascontiguousarray(wh.reshape(ET, 128, vsp).transpose(1, 0, 2))

    sp = np.arange(SC)[:, None]
    tp = np.arange(SC)[None, :]
    mask_h = (sp <= tp).astype(f32)

    nt = t // TB
    nsc = t // SC
    peye_h = np.zeros((128, nt, nt), f32)
    for tb in range(nt):
        peye_h[:, tb, tb] = 1.0
    vtc_h = np.ones((128, nsc, 2), f32)

    return {
        "x0": np.ascontiguousarray(x0),
        "wqkv": np.ascontiguousarray(wqkv_h),
        "wo": np.ascontiguousarray(wo_h),
        "w1": np.ascontiguousarray(w1_h),
        "w2": np.ascontiguousarray(w2_h),
        "vecs": np.ascontiguousarray(vecs_h),
        "fvec": np.ascontiguousarray(fvec_h),
        "whead": whead_h,
        "mask": mask_h,
        "peye": peye_h,
        "vtc": vtc_h,
        "onesr": np.ones((1, 128), f32),
        "onesc": np.ones((HD + 1, 128), f32),
        "selp": np.ascontiguousarray(
            np.broadcast_to(np.eye(nt, dtype=f32)[:, :, None], (nt, nt, 128))
        ),
    }


_NC_CACHE = {}


def _get_nc():
    if "nc" not in _NC_CACHE:
        _NC_CACHE["nc"] = build_nc()
    return _NC_CACHE["nc"]


def kernel(**inputs):
    nc = _get_nc()
    in_maps = [prep_core_inputs(c, **inputs) for c in range(NCORES)]
    res = run_bass_kernel_spmd(nc, in_maps, list(range(NCORES)))
    out = np.empty((B, T, V), np.float32)
    for c in range(NCORES):
        b, j = c // GROUP, c % GROUP
        out[b, :, VS * j : VS * (j + 1)] = res.results[c]["logits"][:, :VS]
    b_head = np.asarray(inputs["b_head"])
    if np.any(b_head):
        out += b_head[None, None, :]
    return out


# revision 30
# speedup vs baseline: 30960.9762x; 1.0166x over previous
"""Trainium2 Bass kernel for a 4-layer bigram-LM dense transformer.

Full-model shapes: B=2, T=2048, E=256, H=8, L=4, V=32000.

Sharding over 8 NeuronCores (self-contained, hardcoded):
  - 2-way data parallel over batch: cores 0-3 handle batch 0, cores 4-7
    batch 1 (a "batch group" of 4 cores each).
  - Within a batch group, per-token work (LN / QKV / wo / FFN) is
    replicated; attention (the exp-heavy part) is sharded 2 heads/core
    and re-assembled with one 4-rank AllGather per layer.
  - lm_head is sharded 4-way over vocab columns within the group
    (8000 cols/core, padded to 8192), so the dominant 524MB logits
    write is split 8 ways.

Compute layout: activations live transposed [E, T] in SBUF so every
matmul contracts over the partition axis with zero transposes. All
matmul operands are float32r (FP22 mantissa truncation, full PE rate
at moving-dim >= 256). Softmax skips the max-subtraction (scores are
~1e-1 scale; exp cannot overflow) and row sums ride along in the
attention-output matmul via a ones column packed next to V, with
normalization folded into the PSUM->SBUF copy.
"""

import numpy as np

import concourse.bass as bass
import concourse.mybir as mybir
import concourse.tile as tile
from concourse import bacc
from concourse.bass_utils import run_bass_kernel_spmd

AF = mybir.ActivationFunctionType
ALU = mybir.AluOpType
FP32 = mybir.dt.float32
FP32R = mybir.dt.float32r

# model dims (full problem)
B, T, E, H, L, V = 2, 2048, 256, 8, 4, 32000
HD = E // H  # 32
EPS = 1e-5
NCORES = 8
GROUP = 4  # cores per batch group
HPC = H // GROUP  # heads per core (2)
VS = V // GROUP  # vocab shard per core (8000)
VSP = 8192  # padded vocab shard
TB = 512  # t-block (PSUM bank free dim)
SC = 128  # s-chunk (partition dim)
ET = E // 128  # embedding partition tiles (2)
FF = 4 * E  # 1024
UT = FF // 128  # ffn u-tiles (8)


def build_nc(t=T, layers=L, vsp=VSP, use_collective=True, ablate=()):
    """Build + compile the per-core Bass program (SPMD: same program, 8 cores)."""
    nt = t // TB      # t-blocks
    nsc = t // SC     # s-chunks
    ntc = t // 128    # t-chunks for lm head
    nvb = vsp // 512  # vocab blocks

    nc = bacc.Bacc("TRN2", num_devices=NCORES)

    # ---- DRAM parameters (per core) ----
    x0 = nc.declare_dram_parameter("x0", [E, t], FP32R, isOutput=False)
    wqkv = nc.declare_dram_parameter("wqkv", [layers, 128, ET, 6 * HD], FP32R, isOutput=False)
    wo_p = nc.declare_dram_parameter("wo", [layers, 128, ET, E], FP32R, isOutput=False)
    w1_p = nc.declare_dram_parameter("w1", [layers, 128, ET, FF], FP32R, isOutput=False)
    w2_p = nc.declare_dram_parameter("w2", [layers, 128, UT, E], FP32R, isOutput=False)
    vecs = nc.declare_dram_parameter("vecs", [layers, 128, 20], FP32, isOutput=False)
    fvec = nc.declare_dram_parameter("fvec", [128, 4], FP32, isOutput=False)
    whead = nc.declare_dram_parameter("whead", [128, ET, vsp], FP32R, isOutput=False)
    maskp = nc.declare_dram_parameter("mask", [SC, SC], FP32, isOutput=False)
    peye = nc.declare_dram_parameter("peye", [128, nt, nt], FP32R, isOutput=False)
    vtc = nc.declare_dram_parameter("vtc", [128, nsc, 2], FP32R, False)
    onesr = nc.declare_dram_parameter("onesr", [1, 128], FP32R, isOutput=False)
    selp = nc.declare_dram_parameter("selp", [nt, nt, 128], FP32R, isOutput=False)
    onesc = nc.declare_dram_parameter("onesc", [HD + 1, 128], FP32R, isOutput=False)
    logits = nc.declare_dram_parameter("logits", [t, vsp], FP32, isOutput=True)

    # internal DRAM bounce buffers for the per-layer AllGather
    cc_in = [nc.dram_tensor(f"cc_in{l}", [HPC * HD, t], FP32R) for l in range(layers)]
    cc_out = [nc.dram_tensor(f"cc_out{l}", [GROUP * HPC * HD, t], FP32R) for l in range(layers)]
    groups = [[0, 1, 2, 3], [4, 5, 6, 7]]

    from contextlib import ExitStack
    with tile.TileContext(nc) as tc:
        with ExitStack() as _ctx:
            persist = _ctx.enter_context(tc.tile_pool(name="persist", bufs=1))
            wpool2 = _ctx.enter_context(tc.tile_pool(name="wpool2", bufs=2))
            wpool1 = _ctx.enter_context(tc.tile_pool(name="wpool1", bufs=1))
            actp = _ctx.enter_context(tc.tile_pool(name="actp", bufs=1))
            xlnp = _ctx.enter_context(tc.tile_pool(name="xlnp", bufs=2))
            bigp = _ctx.enter_context(tc.tile_pool(name="bigp", bufs=3))
            expp = _ctx.enter_context(tc.tile_pool(name="expp", bufs=3))
            smallp = _ctx.enter_context(tc.tile_pool(name="smallp", bufs=2))
            tmpp = _ctx.enter_context(tc.tile_pool(name="tmpp", bufs=2))
            lgp = _ctx.enter_context(tc.tile_pool(name="lgp", bufs=6))
            whp = _ctx.enter_context(tc.tile_pool(name="whp", bufs=3))
            dpool = _ctx.enter_context(tc.tile_pool(name="dpool", bufs=2, space="DRAM"))
            ps_a = _ctx.enter_context(tc.tile_pool(name="ps_a", bufs=2, space="PSUM"))
            ps_o = _ctx.enter_context(tc.tile_pool(name="ps_o", bufs=1, space="PSUM"))
            ps_m = _ctx.enter_context(tc.tile_pool(name="ps_m", bufs=2, space="PSUM"))
            ps_s = _ctx.enter_context(tc.tile_pool(name="ps_s", bufs=1, space="PSUM"))
            # ---- persistent tiles ----
            xT = [persist.tile([128, t], FP32R, tag=f"xT{e}", name=f"xT{e}") for e in range(ET)]
            for e in range(ET):
                nc.sync.dma_start(out=xT[e], in_=x0[128 * e : 128 * (e + 1), :])
            mask = persist.tile([SC, SC], FP32, tag="mask")
            nc.sync.dma_start(out=mask, in_=maskp[:, :])
            fv = persist.tile([128, 4], FP32, tag="fvec")
            nc.sync.dma_start(out=fv, in_=fvec[:, :])
            # v tile: per chunk cols = [vA(32) | ones | vB(32) | ones] so the
            # 33-wide per-head lhsT computes o rows 0:32 plus a row-sum row 32
            vt = persist.tile([128, nsc, 2 * (HD + 1)], FP32R, tag="vt")
            nc.sync.dma_start(out=vt[:, :, HD : HD + 1], in_=vtc[:, :, 0:1])
            nc.sync.dma_start(out=vt[:, :, 2 * HD + 1 : 2 * HD + 2], in_=vtc[:, :, 1:2])
            eyeblk = persist.tile([128, nt, nt], FP32R, tag="eyeblk")
            nc.sync.dma_start(out=eyeblk, in_=peye[:, :, :])
            ones1c = persist.tile([1, 128], FP32R, tag="ones1c")
            nc.sync.dma_start(out=ones1c, in_=onesr[:, :])
            selt = persist.tile([nt, nt, 128], FP32R, tag="selt")
            nc.sync.dma_start(out=selt, in_=selp[:, :, :])
            ones33 = persist.tile([HD + 1, 128], FP32R, tag="ones33")
            nc.sync.dma_start(out=ones33, in_=onesc[:, :])
            # own heads' normalized attention out, pre-AllGather, [32, t] each
            oTp = [persist.tile([HD, t], FP32R, tag=f"oTp{h}", name=f"oTp{h}") for h in range(HPC)]
            epst = persist.tile([128, 1], FP32, tag="epst")
            nc.vector.memset(epst, EPS)

            def layernorm(src, g_ap_of, b_ap_of, out_tiles):
                if "ln" in ablate:
                    for e in range(ET):
                        nc.scalar.activation(
                            out=out_tiles[e][:, :], in_=src[e][:, :], func=AF.Identity,
                            bias=b_ap_of(e), scale=g_ap_of(e),
                        )
                    return
                """src: list of ET [128, t] fp32r tiles -> out_tiles fp32r.

                Per-token stats via ones-matmuls into PSUM rows {0,32,64,96}
                (one per t-block), then (x*s + m2)*g + b with s=rstd,
                m2=-mean*rstd broadcast along partitions.
                """
                sq = [
                    bigp.tile([128, t], FP32R, tag="big", name=f"sq{e}")
                    for e in range(ET)
                ]
                xs_ps = ps_s.tile([nt, TB], FP32, tag="stat_x")
                qs_ps = ps_s.tile([nt, TB], FP32, tag="stat_q")
                for tb in range(nt):
                    tbl = slice(TB * tb, TB * (tb + 1))
                    for e in range(ET):
                        nc.vector.tensor_tensor(
                            out=sq[e][:, tbl], in0=src[e][:, tbl],
                            in1=src[e][:, tbl], op=ALU.mult,
                        )
                    for e in range(ET):
                        nc.tensor.matmul(
                            xs_ps[:, :],
                            eyeblk[:, tb, :],
                            src[e][:, tbl],
                            start=(tb == 0 and e == 0),
                            stop=(tb == nt - 1 and e == ET - 1),
                        )
                    for e in range(ET):
                        nc.tensor.matmul(
                            qs_ps[:, :],
                            eyeblk[:, tb, :],
                            sq[e][:, tbl],
                            start=(tb == 0 and e == 0),
                            stop=(tb == nt - 1 and e == ET - 1),
                        )
                mean4 = smallp.tile([nt, TB], FP32, tag="mean4", name="mean4")
                msq4 = smallp.tile([nt, TB], FP32, tag="msq4", name="msq4")
                var4 = smallp.tile([nt, TB], FP32, tag="var4", name="var4")
                s4 = smallp.tile([nt, TB], FP32R, tag="s4", name="s4")
                xs_rows = xs_ps[:, :]
                qs_rows = qs_ps[:, :]
                nc.vector.tensor_scalar(mean4[:, :], xs_rows, 1.0 / E, None, ALU.mult)
                nc.vector.tensor_scalar(msq4[:, :], qs_rows, 1.0 / E, None, ALU.mult)
                nc.vector.tensor_tensor(
                    out=var4[:, :], in0=mean4[:, :], in1=mean4[:, :], op=ALU.mult
                )
                nc.vector.tensor_tensor(
                    out=var4[:, :], in0=msq4[:, :], in1=var4[:, :], op=ALU.subtract
                )
                nc.scalar.activation(out=var4[:, :], in_=var4[:, :], func=AF.Ln, bias=epst[0:nt, :])
                nc.scalar.activation(out=s4[:, :], in_=var4[:, :], func=AF.Exp, scale=-0.5)
                m24 = smallp.tile([nt, TB], FP32R, tag="msq4", name="m24")
                nc.vector.scalar_tensor_tensor(
                    out=m24[:, :], in0=mean4[:, :], scalar=-1.0, in1=s4[:, :],
                    op0=ALU.mult, op1=ALU.mult,
                )
                for tb in range(nt):
                    s_bc = ps_s.tile([128, TB], FP32, tag="stat_x", name="s_bc")
                    m_bc = ps_s.tile([128, TB], FP32, tag="stat_q", name="m_bc")
                    nc.tensor.matmul(
                        s_bc[:, :], selt[:, tb, :], s4[:, :],
                        start=True, stop=True,
                    )
                    nc.tensor.matmul(
                        m_bc[:, :], selt[:, tb, :], m24[:, :],
                        start=True, stop=True,
                    )
                    for e in range(ET):
                        tmp = tmpp.tile([128, TB], FP32, tag="lntmp")
                        nc.vector.tensor_tensor(
                            out=tmp,
                            in0=src[e][:, TB * tb : TB * (tb + 1)],
                            in1=s_bc[:, :], op=ALU.mult,
                        )
                        nc.vector.tensor_tensor(
                            out=tmp, in0=tmp, in1=m_bc[:, :], op=ALU.add,
                        )
                        nc.scalar.activation(
                            out=out_tiles[e][:, TB * tb : TB * (tb + 1)],
                            in_=tmp, func=AF.Identity,
                            bias=b_ap_of(e), scale=g_ap_of(e),
                        )

            # ================= layers =================
            for l in range(layers):
                wq_t = [wpool2.tile([128, 6 * HD], FP32R, tag=f"wqkv{e}", name=f"wqkv{e}") for e in range(ET)]
                wo_t = [wpool2.tile([128, E], FP32R, tag=f"wo{e}", name=f"wot{e}") for e in range(ET)]
                w1_t = [wpool1.tile([128, FF], FP32R, tag=f"w1{e}", name=f"w1t{e}") for e in range(ET)]
                w2_t = wpool1.tile([128, UT, E], FP32R, tag="w2")
                vec = wpool2.tile([128, 20], FP32, tag="vec")
                for e in range(ET):
                    nc.sync.dma_start(out=wq_t[e], in_=wqkv[l, :, e, :])
                    nc.sync.dma_start(out=wo_t[e], in_=wo_p[l, :, e, :])
                    nc.sync.dma_start(out=w1_t[e], in_=w1_p[l, :, e, :])
                nc.sync.dma_start(out=w2_t, in_=w2_p[l, :, :, :])
                nc.sync.dma_start(out=vec, in_=vecs[l, :, :])

                xln = [xlnp.tile([128, t], FP32R, tag=f"xln{e}", name=f"xln{e}") for e in range(ET)]
                layernorm(
                    xT,
                    g_ap_of=lambda e: vec[:, 0 + e : 1 + e],
                    b_ap_of=lambda e: vec[:, 2 + e : 3 + e],
                    out_tiles=xln,
                )

                qT = actp.tile([2 * HD, t], FP32R, tag="qT")
                kT = actp.tile([2 * HD, t], FP32R, tag="kT")
                for tb in range(nt):
                    tsl = slice(TB * tb, TB * (tb + 1))
                    qp = ps_m.tile([2 * HD, TB], FP32, tag="m")
                    for e in range(ET):
                        nc.tensor.matmul(
                            qp[:, :], wq_t[e][:, 0 : 2 * HD], xln[e][:, tsl],
                            start=(e == 0), stop=(e == ET - 1),
                        )
                    nc.any.tensor_copy(out=qT[:, tsl], in_=qp[:, :])
                    kp = ps_a.tile([2 * HD, TB], FP32, tag="att", name="kp")
                    for e in range(ET):
                        nc.tensor.matmul(
                            kp[:, :], wq_t[e][:, 2 * HD : 4 * HD], xln[e][:, tsl],
                            start=(e == 0), stop=(e == ET - 1),
                        )
                    nc.any.tensor_copy(out=kT[:, tsl], in_=kp[:, :])
                    for i in range(4 * tb, 4 * tb + 4):
                        vp = ps_a.tile([128, 2 * HD], FP32, tag="att", name="vp")
                        for e in range(ET):
                            nc.tensor.matmul(
                                vp[:, :],
                                xln[e][:, SC * i : SC * (i + 1)],
                                wq_t[e][:, 4 * HD : 6 * HD],
                                start=(e == 0), stop=(e == ET - 1),
                            )
                        nc.any.tensor_copy(out=vt[:, i, 0:HD], in_=vp[:, 0:HD])
                        nc.any.tensor_copy(
                            out=vt[:, i, HD + 1 : 2 * HD + 1], in_=vp[:, HD : 2 * HD]
                        )

                    # ---- attention for this t-block (qkv ready up to here) ----
                    if "attn" in ablate:
                        if tb == 0:
                            for h in range(HPC):
                                nc.vector.memset(oTp[h].bitcast(FP32), 1.0)
                        continue
                    op_ps = [
                        ps_o.tile([HD + 1, TB], FP32, tag=f"o{h}", name=f"op_ps{h}")
                        for h in range(HPC)
                    ]
                    nmax = 4 * tb + 4
                    for h in range(HPC):
                        rsl = slice(32 * h, 32 * (h + 1))

                        def emit_o(i, exh, d):
                            nc.tensor.matmul(
                                op_ps[h][:, d:TB],
                                vt[:, i, (HD + 1) * h : (HD + 1) * h + HD + 1],
                                exh[:, d:TB],
                                start=(i == 0), stop=(i == nmax - 1),
                            )

                        pend = None
                        for i in range(nmax):
                            d = max(0, SC * i - TB * tb)
                            psl = slice(d, TB)
                            tgl = slice(TB * tb + d, TB * (tb + 1))
                            at_ps = ps_a.tile([128, TB], FP32, tag="att", name="at_ps")
                            exh = expp.tile([128, TB], FP32R, tag=f"exp{h}", name="exh")
                            nc.tensor.matmul(
                                at_ps[:, psl],
                                kT[rsl, SC * i : SC * (i + 1)],
                                qT[rsl, tgl],
                                start=True, stop=True,
                                tile_position=(32 * h, 0),
                            )
                            nc.scalar.activation(
                                out=exh[:, psl], in_=at_ps[:, psl],
                                func=AF.Exp, scale=float(E) ** -0.5,
                            )
                            if i >= 4 * tb:  # diagonal chunk: mask upper triangle
                                nc.vector.tensor_tensor(
                                    out=exh[:, d : d + SC],
                                    in0=exh[:, d : d + SC],
                                    in1=mask[:, :], op=ALU.mult,
                                )
                            if pend is not None:
                                emit_o(*pend)
                            pend = (i, exh, d)
                        emit_o(*pend)
                    # normalize each head by its row-sum (psum row 32)
                    srow = [
                        smallp.tile([HD + 1, TB], FP32, tag=f"srow{h}", name=f"srow{h}")
                        for h in range(HPC)
                    ]
                    rd = dpool.tile([HPC, TB], FP32, tag="rd", name="rd")
                    for h in range(HPC):
                        nc.vector.reciprocal(
                            out=srow[h][HD : HD + 1, :],
                            in_=op_ps[h][HD : HD + 1, :],
                        )
                        nc.sync.dma_start(
                            out=rd[h : h + 1, :],
                            in_=srow[h][HD : HD + 1, :],
                        )
                    rec_bc = tmpp.tile([HD, TB], FP32, tag="rbc", name="rec_bc")
                    for h in range(HPC):
                        nc.gpsimd.dma_start(
                            out=rec_bc,
                            in_=rd[h : h + 1, :].partition_broadcast(HD),
                        )
                        nc.vector.tensor_tensor(
                            out=oTp[h][:, TB * tb : TB * (tb + 1)],
                            in0=op_ps[h][0:HD, :],
                            in1=rec_bc,
                            op=ALU.mult,
                        )

                # ---- AllGather heads across the 4-core batch group ----
                oT = [actp.tile([128, t], FP32R, tag=tg, name=f"oT_{tg}") for tg in ("qT", "kT")]
                if use_collective:
                    for h in range(HPC):
                        nc.sync.dma_start(
                            out=cc_in[l][HD * h : HD * (h + 1), :], in_=oTp[h][:, :]
                        )
                    nc.gpsimd.collective_compute(
                        "AllGather", ALU.bypass,
                        replica_groups=groups,
                        ins=[cc_in[l][:, :]], outs=[cc_out[l][:, :]],
                    )
                    for e in range(ET):
                        nc.sync.dma_start(
                            out=oT[e], in_=cc_out[l][128 * e : 128 * (e + 1), :]
                        )
                else:  # single-group debug path (no comm): own heads only
                    for e in range(ET):
                        nc.vector.memset(oT[e].bitcast(FP32), 0.0)
                    for h in range(HPC):
                        nc.vector.tensor_copy(
                            out=oT[0][HD * h : HD * (h + 1), :].bitcast(FP32),
                            in_=oTp[h][:, :].bitcast(FP32),
                        )

                # ---- wo projection + residual ----
                for tb in range(nt):
                    tsl = slice(TB * tb, TB * (tb + 1))
                    for eo in range(ET):
                        wp = ps_m.tile([128, TB], FP32, tag="m")
                        for e in range(ET):
                            nc.tensor.matmul(
                                wp[:, :],
                                wo_t[e][:, 128 * eo : 128 * (eo + 1)],
                                oT[e][:, tsl],
                                start=(e == 0), stop=(e == ET - 1),
                            )
                        nc.vector.scalar_tensor_tensor(
                            out=xT[eo][:, tsl], in0=wp[:, :],
                            scalar=vec[:, 8 + eo : 9 + eo], in1=xT[eo][:, tsl],
                            op0=ALU.add, op1=ALU.add,
                        )

                # ---- FFN ----
                xln2 = [xlnp.tile([128, t], FP32R, tag=f"xln{e}", name=f"xln{e}") for e in range(ET)]
                layernorm(
                    xT,
                    g_ap_of=lambda e: vec[:, 4 + e : 5 + e],
                    b_ap_of=lambda e: vec[:, 6 + e : 7 + e],
                    out_tiles=xln2,
                )
                for tb in range(nt):
                    if "ffn" in ablate:
                        break
                    tsl = slice(TB * tb, TB * (tb + 1))
                    ru_halves = []
                    for half in range(2):
                        ru = bigp.tile([128, UT // 2, TB], FP32R, tag="big", name="ru")
                        for uu in range(UT // 2):
                            ut = half * (UT // 2) + uu
                            up = ps_a.tile([128, TB], FP32, tag="att", name="up")
                            for e in range(ET):
                                nc.tensor.matmul(
                                    up[:, :],
                                    w1_t[e][:, 128 * ut : 128 * (ut + 1)],
                                    xln2[e][:, tsl],
                                    start=(e == 0), stop=(e == ET - 1),
                                )
                            nc.scalar.activation(
                                out=ru[:, uu, :], in_=up[:, :], func=AF.Relu,
                                bias=vec[:, 10 + ut : 11 + ut],
                            )
                        ru_halves.append(ru)
                    for eo in range(ET):
                        wp2 = ps_m.tile([128, TB], FP32, tag="m", name="wp2")
                        for ut in range(UT):
                            nc.tensor.matmul(
                                wp2[:, :],
                                w2_t[:, ut, 128 * eo : 128 * (eo + 1)],
                                ru_halves[ut // (UT // 2)][:, ut % (UT // 2), :],
                                start=(ut == 0), stop=(ut == UT - 1),
                            )
                        nc.vector.scalar_tensor_tensor(
                            out=xT[eo][:, tsl], in0=wp2[:, :],
                            scalar=vec[:, 18 + eo : 19 + eo], in1=xT[eo][:, tsl],
                            op0=ALU.add, op1=ALU.add,
                        )

            # ================= final LN + lm_head =================
            xf = [xlnp.tile([128, t], FP32R, tag=f"xln{e}", name=f"xln{e}") for e in range(ET)]
            layernorm(
                xT,
                g_ap_of=lambda e: fv[:, 0 + e : 1 + e],
                b_ap_of=lambda e: fv[:, 2 + e : 3 + e],
                out_tiles=xf,
            )
            for vb in range(nvb if "lm" not in ablate else 1):
                wh = whp.tile([128, ET, 512], FP32R, tag="wh")
                nc.sync.dma_start(out=wh, in_=whead[:, :, 512 * vb : 512 * (vb + 1)])
                for tcn in range(ntc):
                    lp = ps_m.tile([128, 512], FP32, tag="m")
                    for e in range(ET):
                        nc.tensor.matmul(
                            lp[:, :],
                            xf[e][:, 128 * tcn : 128 * (tcn + 1)],
                            wh[:, e, :],
                            start=(e == 0), stop=(e == ET - 1),
                        )
                    lg = lgp.tile([128, 512], FP32, tag="lg")
                    if (vb + tcn) % 2 == 0:
                        nc.vector.tensor_copy(out=lg, in_=lp[:, :])
                    else:
                        nc.scalar.copy(out=lg, in_=lp[:, :])
                    nc.sync.dma_start(
                        out=logits[128 * tcn : 128 * (tcn + 1), 512 * vb : 512 * (vb + 1)],
                        in_=lg,
                    )

    nc.compile()
    return nc


# ---------------- host-side prep / unshard ----------------

def prep_core_inputs(c, X, tok_emb, pos_emb, wq, wk, wv, wo, bo, w1, b1, w2, b2,
                     ln1_g, ln1_b, ln2_g, ln2_b, lnf_g, lnf_b, w_head, b_head,
                     t=T, layers=L, vsp=VSP):
    b = c // GROUP
    j = c % GROUP
    heads = [HPC * j + k for k in range(HPC)]

    f32 = np.float32
    Xb = np.asarray(X[b]).astype(np.int64)
    x0 = (np.asarray(tok_emb)[Xb] + np.asarray(pos_emb)[:t]).astype(f32).T  # [E, t]

    wq = np.asarray(wq); wk = np.asarray(wk); wv = np.asarray(wv)
    wqkv_h = np.empty((layers, 128, ET, 6 * HD), f32)
    wo_h = np.empty((layers, 128, ET, E), f32)
    w1_h = np.empty((layers, 128, ET, FF), f32)
    w2_h = np.empty((layers, 128, UT, E), f32)
    vecs_h = np.empty((layers, 128, 20), f32)
    for l in range(layers):
        qc = np.concatenate([wq[l, h] for h in heads], axis=1)  # [E, 64]
        kc = np.concatenate([wk[l, h] for h in heads], axis=1)
        vc = np.concatenate([wv[l, h] for h in heads], axis=1)
        qkv = np.concatenate([qc, kc, vc], axis=1)  # [E, 192]
        wqkv_h[l] = qkv.reshape(ET, 128, 6 * HD).transpose(1, 0, 2)
        wo_h[l] = np.asarray(wo[l]).reshape(ET, 128, E).transpose(1, 0, 2)
        w1_h[l] = np.asarray(w1[l]).reshape(ET, 128, FF).transpose(1, 0, 2)
        w2_h[l] = np.asarray(w2[l]).reshape(UT, 128, E).transpose(1, 0, 2)
        vv = np.concatenate([
            np.asarray(ln1_g[l]), np.asarray(ln1_b[l]),
            np.asarray(ln2_g[l]), np.asarray(ln2_b[l]),
            np.asarray(bo[l]), np.asarray(b1[l]), np.asarray(b2[l]),
        ]).astype(f32)  # 2560
        vecs_h[l] = vv.reshape(20, 128).T
    fvec_h = np.concatenate(
        [np.asarray(lnf_g), np.asarray(lnf_b)]
    ).astype(f32).reshape(4, 128).T

    w_head = np.asarray(w_head)
    vs = w_head.shape[1] // GROUP
    wh = np.zeros((E, vsp), f32)
    wh[:, :vs] = w_head[:, vs * j : vs * (j + 1)]
    whead_h = np.ascontiguousarray(wh.reshape(ET, 128, vsp).transpose(1, 0, 2))

    sp = np.arange(SC)[:, None]
    tp = np.arange(SC)[None, :]
    mask_h = (sp <= tp).astype(f32)

    nt = t // TB
    nsc = t // SC
    peye_h = np.zeros((128, nt, nt), f32)
    for tb in range(nt):
        peye_h[:, tb, tb] = 1.0
    vtc_h = np.ones((128, nsc, 2), f32)

    return {
        "x0": np.ascontiguousarray(x0),
        "wqkv": np.ascontiguousarray(wqkv_h),
        "wo": np.ascontiguousarray(wo_h),
        "w1": np.ascontiguousarray(w1_h),
        "w2": np.ascontiguousarray(w2_h),
        "vecs": np.ascontiguousarray(vecs_h),
        "fvec": np.ascontiguousarray(fvec_h),
        "whead": whead_h,
        "mask": mask_h,
        "peye": peye_h,
        "vtc": vtc_h,
        "onesr": np.ones((1, 128), f32),
        "onesc": np.ones((HD + 1, 128), f32),
        "selp": np.ascontiguousarray(
            np.broadcast_to(np.eye(nt, dtype=f32)[:, :, None], (nt, nt, 128))
        ),
    }


_NC_CACHE = {}


def _get_nc():
    if "nc" not in _NC_CACHE:
        _NC_CACHE["nc"] = build_nc()
    return _NC_CACHE["nc"]


def kernel(**inputs):
    nc = _get_nc()
    in_maps = [prep_core_inputs(c, **inputs) for c in range(NCORES)]
    res = run_bass_kernel_spmd(nc, in_maps, list(range(NCORES)))
    out = np.empty((B, T, V), np.float32)
    for c in range(NCORES):
        b, j = c // GROUP, c % GROUP
        out[b, :, VS * j : VS * (j + 1)] = res.results[c]["logits"][:, :VS]
    b_head = np.asarray(inputs["b_head"])
    if np.any(b_head):
        out += b_head[None, None, :]
    return out
